# revision 9
# baseline (speedup 1.0000x reference)
"""Bilateral filter (5x5, reflect pad) on 8 Trainium2 NeuronCores.

Contract: kernel(**inputs) takes the FULL inputs
  x:              [4, 3, 512, 512] f32
  spatial_kernel: [5, 5] f32
  sigma_color:    scalar f32
and returns the FULL output [4, 3, 512, 512] f32.

v2: paired-plane algorithm. The bilateral weight between two pixels is
symmetric, so the 24 non-center taps form 12 point-symmetric pairs:
  w_refl[i,j] = w_rep[i-di', j-dj']      (di' = di-2, dj' = dj-2)
  u_refl[i,j] = -u_rep[i-di', j-dj']     (u = w * d, d = tap - center)
Only 12 "representative" planes (di in {3,4} x dj, plus (2,3),(2,4)) are
computed (halves ACT + DVE work vs all 24). Reductions
  S = 1 + sum_t wsk_t (w_t + w_t_shifted)
  U =     sum_t wsk_t (u_t - u_t_shifted)
run on the PE as 51 matmuls/tile: scaled identities for direct terms,
scaled shifted identities (superdiagonal) for row-shifted terms, and two
tiny seam matmuls for the 2 top rows whose shifted reads fall into the
previous tile (strips gathered by SBUF-SBUF DMA from the previous tile's
planes; chain-first tiles get host-computed strips). Output:
  out = center + U * reciprocal(S).

Sharding: each core gets one full image (512-row chain of 4 tiles) plus
one lone half image (256-row chain of 2 tiles) as three [260, 520] fp16
pieces (rows +-2, cols +-4 reflect halo), converted to fp16 on the host.
"""

import os

import numpy as np

import bass_rust
import concourse.bacc as bacc
import concourse.bass as bass
import concourse.mybir as mybir
import concourse.tile as tile
from concourse import bass_utils

F32 = mybir.dt.float32
FP16 = mybir.dt.float16
AF = mybir.ActivationFunctionType
ALU = mybir.AluOpType

N_CORES = 8
K = 5
B, C, H, W = 4, 3, 512, 512
N_IMGS = B * C                    # 12
HALF = 256
PIECE_ROWS = HALF + 4             # 260 (rows +-2)
PIECE_COLS = W + 8                # 520 (cols +-4)
PLANE_COLS = W + 4                # 516
NSLOT = 12

# plane slots: 0..4 = (3, 0..4), 5..9 = (4, 0..4), 10 = (2,3), 11 = (2,4)
REPS = [(3, d) for d in range(5)] + [(4, d) for d in range(5)] + [(2, 3), (2, 4)]
CLS_VALS = [1, 2, 4, 5, 8]        # distinct (di-2)^2 + (dj-2)^2 of reps
CLS_OF = [CLS_VALS.index((di - 2) ** 2 + (dj - 2) ** 2) for di, dj in REPS]

# lhsT pack layout (22 matrices of [128, 128] fp16)
LT_ONES = 0                       # row 0 = ones (center tap: S += 1)
LT_A = 1                          # +wsk_c * I          (5: cls 0..4)
LT_B = 6                          # -wsk_c * I, c in {1, 4} -> cls idx {0, 2}
LT_Z1 = 8                         # +wsk_c * Z1, c in {1,2,5} -> cls {0,1,3}
LT_Z2 = 11                        # +wsk_c * Z2, c in {4,5,8} -> cls {2,3,4}
LT_N1 = 14                        # -wsk_c * Z1
LT_N2 = 17                        # -wsk_c * Z2
LT_SEAM_S = 20
LT_SEAM_U = 21
N_MAT = 22
B_IDX = {0: 6, 2: 7}              # cls idx -> B slot
Z1_IDX = {0: 8, 1: 9, 3: 10}
Z2_IDX = {2: 11, 3: 12, 4: 13}
N1_IDX = {0: 14, 1: 15, 3: 16}
N2_IDX = {2: 17, 3: 18, 4: 19}

SUB_GROUPS = [(3, 0, 0, 5), (4, 0, 5, 5), (2, 3, 10, 2)]  # (di, dj0, slot0, g)

PLANE_EXT = NSLOT * PLANE_COLS    # 6192 elems per partition in W/U planes
NORM = float(2.0 / np.sqrt(np.pi))   # Derivative_Erf amplitude

_cached = {}


def _build(wsk_cls: dict, gamma: float) -> bass.Bass:
    """Per-core Bass module (SPMD: same NEFF on all 8 cores)."""
    nc = bacc.Bacc("TRN2", target_bir_lowering=False, debug=False)
    x_in = nc.dram_tensor(
        "x_in", [3, PIECE_ROWS, PIECE_COLS], FP16, kind="ExternalInput"
    ).ap()
    strips_in = nc.dram_tensor(
        "strips", [2, 2, 2, NSLOT, PLANE_COLS], FP16, kind="ExternalInput"
    ).ap()
    lhst_in = nc.dram_tensor(
        "lhst", [128, N_MAT, 128], FP16, kind="ExternalInput"
    ).ap()
    y_out = nc.dram_tensor(
        "y_out", [3, HALF, W], F32, kind="ExternalOutput"
    ).ap()

    with tile.TileContext(nc) as tc:
        with (
            tc.tile_pool(name="const_pool", bufs=1) as const_pool,
            tc.tile_pool(name="slab_pool", bufs=2) as slab_pool,
            tc.tile_pool(name="d_pool", bufs=2) as d_pool,
            tc.tile_pool(name="w_pool", bufs=2) as w_pool,
            tc.tile_pool(name="u_pool", bufs=2) as u_pool,
            tc.tile_pool(name="seam_pool", bufs=2) as seam_pool,
            tc.tile_pool(name="epi_pool", bufs=2) as epi_pool,
            tc.tile_pool(name="psum_pool", bufs=2, space="PSUM") as psum_pool,
        ):
            lhst = const_pool.tile([128, N_MAT, 128], FP16, tag="lhst",
                                   name="lhst")
            nc.sync.dma_start(lhst[:, :, :], lhst_in)
            ones_row = const_pool.tile([1, W], FP16, tag="ones", name="ones")
            nc.gpsimd.memset(ones_row[:, :], 1.0)

            # PE warmup: dummy matmuls depending only on the memset keep the
            # PE HAM busy through the first tile's plane production, so the
            # real burst starts at full clock (2.4 GHz, not cold 1.2).
            warm_ps = psum_pool.tile([128, W], F32, tag="warm", name="warm")
            for wi in range(14):
                nc.tensor.matmul(warm_ps[:, :], ones_row[0:1, 0:128],
                                 ones_row[0:1, :], start=(wi == 0),
                                 stop=(wi == 13))

            # tiles: (piece, r0, chain_first, chain_idx)
            tiles = [
                (0, 0, True, 0), (0, 128, False, 0),
                (1, 0, False, 0), (1, 128, False, 0),
                (2, 0, True, 1), (2, 128, False, 1),
            ]
            st = {}

            def prod(ti):
                """Plane production for tile ti: slab DMA, seam gathers,
                d -> w -> u, col-pair presums."""
                pc, r0, first, ch = tiles[ti]
                slab = slab_pool.tile([128, 3, PIECE_COLS], FP16, tag="slab",
                                      name=f"slab{ti}")
                src = x_in[pc, r0 + 2 : r0 + 2 + 128, :].copy()
                src.ap = bass_rust.VecI64Pair(
                    [(PIECE_COLS, 128), (PIECE_COLS, 3), (1, PIECE_COLS)]
                )
                nc.sync.dma_start(slab[:, :, :], src)
                slab_base = slab[:, :, :].offset

                seamW = seam_pool.tile([15, W], FP16, tag="sw", name=f"sw{ti}")
                seamU = seam_pool.tile([15, W], FP16, tag="su", name=f"su{ti}")
                for dst, which in ((seamW, 0), (seamU, 1)):
                    if first:
                        base = ch * (2 * 2 * PLANE_EXT) + which * (2 * PLANE_EXT)
                        proto = strips_in
                        off0 = base + 1 * PLANE_EXT + 4          # row ps=-1
                        off1 = base + 0 * PLANE_EXT + 5 * PLANE_COLS + 4
                    else:
                        prev = st[ti - 1]["W"] if which == 0 else st[ti - 1]["U"]
                        proto = prev[:, :, :]
                        pbase = proto.offset
                        off0 = pbase + 127 * PLANE_EXT + 4       # prev row 127
                        off1 = pbase + 126 * PLANE_EXT + 5 * PLANE_COLS + 4
                    v = proto.copy()
                    v.ap = bass_rust.VecI64Pair(
                        [(PLANE_EXT, 1), (PLANE_COLS - 1, 5), (1, W)]
                    )
                    v.offset = off0
                    nc.sync.dma_start(dst[0:5, :], v)
                    v2 = proto.copy()
                    v2.ap = bass_rust.VecI64Pair(
                        [(PLANE_EXT, 2), (PLANE_COLS - 1, 5), (1, W)]
                    )
                    v2.offset = off1
                    nc.sync.dma_start(dst[5:15, :], v2)

                d_all = d_pool.tile([128, NSLOT, PLANE_COLS], FP16, tag="d",
                                    name=f"d{ti}")
                W_all = w_pool.tile([128, NSLOT, PLANE_COLS], FP16, tag="w",
                                    name=f"w{ti}")
                U_all = u_pool.tile([128, NSLOT, PLANE_COLS], FP16, tag="u",
                                    name=f"u{ti}")
                for di, dj0, s0, g in SUB_GROUPS:
                    tap = slab[:, :, :].copy()
                    tap.ap = bass_rust.VecI64Pair(
                        [(3 * PIECE_COLS, 128), (1, g), (1, PLANE_COLS)]
                    )
                    tap.offset = slab_base + (di - 2) * PIECE_COLS + dj0
                    cen = slab[:, :, :].copy()
                    cen.ap = bass_rust.VecI64Pair(
                        [(3 * PIECE_COLS, 128), (0, g), (1, PLANE_COLS)]
                    )
                    cen.offset = slab_base + 2
                    nc.vector.tensor_sub(d_all[:, s0 : s0 + g, :], tap, cen)
                    nc.scalar.activation(W_all[:, s0 : s0 + g, :],
                                         d_all[:, s0 : s0 + g, :],
                                         AF.Derivative_Erf, scale=float(gamma))
                    nc.vector.tensor_mul(U_all[:, s0 : s0 + g, :],
                                         W_all[:, s0 : s0 + g, :],
                                         d_all[:, s0 : s0 + g, :])

                vt = seam_pool.tile([128, 2, W], FP16, tag="v", name=f"v{ti}")
                zt = seam_pool.tile([128, 2, W], FP16, tag="z", name=f"z{ti}")
                wsh = W_all[:, :, :].copy()
                wsh.ap = bass_rust.VecI64Pair(
                    [(PLANE_EXT, 128), (PLANE_COLS - 1, 2), (1, W)]
                )
                wsh.offset = W_all[:, :, :].offset + 10 * PLANE_COLS + 1
                nc.vector.tensor_add(vt[:, :, :], W_all[:, 10:12, 2 : 2 + W],
                                     wsh)
                ush = U_all[:, :, :].copy()
                ush.ap = bass_rust.VecI64Pair(
                    [(PLANE_EXT, 128), (PLANE_COLS - 1, 2), (1, W)]
                )
                ush.offset = U_all[:, :, :].offset + 10 * PLANE_COLS + 1
                nc.vector.tensor_sub(zt[:, :, :], U_all[:, 10:12, 2 : 2 + W],
                                     ush)
                st[ti] = dict(slab=slab, slab_base=slab_base, seamW=seamW,
                              seamU=seamU, W=W_all, U=U_all, vt=vt, zt=zt)

            def burst_epi(ti):
                pc, r0, first, ch = tiles[ti]
                s_ = st[ti]
                W_all, U_all = s_["W"], s_["U"]
                S_ps = psum_pool.tile([128, W], F32, tag="S", name=f"S{ti}")
                U_ps = psum_pool.tile([128, W], F32, tag="U", name=f"U{ti}")
                for s in range(10):
                    c = CLS_OF[s]
                    a = 4 - REPS[s][1]
                    sshift = Z1_IDX[c] if s < 5 else Z2_IDX[c]
                    ushift = N1_IDX[c] if s < 5 else N2_IDX[c]
                    nc.tensor.matmul(S_ps[:, :], lhst[:, LT_A + c, :],
                                     W_all[:, s, 2 : 2 + W],
                                     start=(s == 0), stop=False)
                    nc.tensor.matmul(U_ps[:, :], lhst[:, LT_A + c, :],
                                     U_all[:, s, 2 : 2 + W],
                                     start=(s == 0), stop=False)
                    nc.tensor.matmul(S_ps[:, :], lhst[:, sshift, :],
                                     W_all[:, s, a : a + W],
                                     start=False, stop=False)
                    nc.tensor.matmul(U_ps[:, :], lhst[:, ushift, :],
                                     U_all[:, s, a : a + W],
                                     start=False, stop=False)
                for i, s in enumerate((10, 11)):
                    c = CLS_OF[s]
                    nc.tensor.matmul(S_ps[:, :], lhst[:, LT_A + c, :],
                                     s_["vt"][:, i, :], start=False, stop=False)
                    nc.tensor.matmul(U_ps[:, :], lhst[:, LT_A + c, :],
                                     s_["zt"][:, i, :], start=False, stop=False)
                nc.tensor.matmul(S_ps[:, :], lhst[0:15, LT_SEAM_S, :],
                                 s_["seamW"][0:15, :], start=False, stop=False)
                nc.tensor.matmul(S_ps[:, :], lhst[0:1, LT_ONES, :],
                                 ones_row[0:1, :], start=False, stop=True)
                nc.tensor.matmul(U_ps[:, :], lhst[0:15, LT_SEAM_U, :],
                                 s_["seamU"][0:15, :], start=False, stop=True)

                R = epi_pool.tile([128, W], F32, tag="R", name=f"R{ti}")
                nc.vector.reciprocal_approx_fast(R[:, :], S_ps[:, :])
                UR = epi_pool.tile([128, W], F32, tag="UR", name=f"UR{ti}")
                nc.vector.scalar_tensor_tensor(UR[:, :], U_ps[:, :], 1.0,
                                               R[:, :], ALU.mult, ALU.mult)
                out_t = epi_pool.tile([128, W], F32, tag="out", name=f"o{ti}")
                cen = s_["slab"][:, :, :].copy()
                cen.ap = bass_rust.VecI64Pair([(3 * PIECE_COLS, 128), (1, W)])
                cen.offset = s_["slab_base"] + 4
                nc.vector.tensor_add(out_t[:, :], UR[:, :], cen)
                nc.sync.dma_start(y_out[pc, r0 : r0 + 128, :], out_t[:, :])

            # software-pipelined emission: production runs one tile ahead
            prod(0)
            for ti in range(6):
                if ti + 1 < 6:
                    prod(ti + 1)
                burst_epi(ti)
    nc.compile()
    return nc


def _make_lhst(wsk_cls: dict) -> np.ndarray:
    """[128, N_MAT, 128] fp16 lhsT pack."""
    mats = np.zeros((N_MAT, 128, 128), dtype=np.float32)
    eye = np.eye(128, dtype=np.float32)
    z1 = np.zeros((128, 128), dtype=np.float32)
    z1[np.arange(127), np.arange(1, 128)] = 1.0   # Z1[p, p+1] = 1
    z2 = np.zeros((128, 128), dtype=np.float32)
    z2[np.arange(126), np.arange(2, 128)] = 1.0
    mats[LT_ONES, 0, :] = 1.0
    for ci in range(5):
        mats[LT_A + ci] = wsk_cls[ci] * eye
    for ci, sl in B_IDX.items():
        mats[sl] = -wsk_cls[ci] * eye
    for ci, sl in Z1_IDX.items():
        mats[sl] = wsk_cls[ci] * z1
    for ci, sl in Z2_IDX.items():
        mats[sl] = wsk_cls[ci] * z2
    for ci, sl in N1_IDX.items():
        mats[sl] = -wsk_cls[ci] * z1
    for ci, sl in N2_IDX.items():
        mats[sl] = -wsk_cls[ci] * z2
    # seam lhsTs: rows 0..4 (dlt_i=1 pairs, slots 0..4) -> out row 0;
    # rows 5..9 (dlt_i=2, ps=-2) -> out row 0; rows 10..14 (ps=-1) -> row 1
    for r in range(5):
        mats[LT_SEAM_S, r, 0] = wsk_cls[CLS_OF[r]]
        mats[LT_SEAM_U, r, 0] = -wsk_cls[CLS_OF[r]]
        mats[LT_SEAM_S, 5 + r, 0] = wsk_cls[CLS_OF[5 + r]]
        mats[LT_SEAM_U, 5 + r, 0] = -wsk_cls[CLS_OF[5 + r]]
        mats[LT_SEAM_S, 10 + r, 1] = wsk_cls[CLS_OF[5 + r]]
        mats[LT_SEAM_U, 10 + r, 1] = -wsk_cls[CLS_OF[5 + r]]
    return np.ascontiguousarray(
        np.transpose(mats, (1, 0, 2)).astype(np.float16)
    )


def _strip_planes(piece: np.ndarray, gamma: float) -> np.ndarray:
    """Host chain-first strips: [2(w/u), 2(row: ps=-2,-1), 12, 516] fp16.
    piece: [260, 520] f32. Matches device plane values (incl. 2/sqrt(pi))."""
    out = np.zeros((2, 2, NSLOT, PLANE_COLS), dtype=np.float32)
    cols = np.arange(PLANE_COLS)
    for s, (di, dj) in enumerate(REPS):
        for ri, p in enumerate((-2, -1)):
            tap = piece[p + di, cols + dj]
            cen = piece[p + 2, cols + 2]
            d = (tap - cen).astype(np.float32)
            w = NORM * np.exp(-(gamma**2) * d * d)
            out[0, ri, s] = w
            out[1, ri, s] = w * d
    return out.astype(np.float16)


def _get_nc(sk: np.ndarray, gamma: float):
    key = (sk.tobytes(), float(gamma))
    if _cached.get("key") != key:
        wsk_cls = {}
        for s, (di, dj) in enumerate(REPS):
            # fold sqrt(pi)/2 so that wsk * D_ERF = sk * exp(-g^2 d^2)
            wsk_cls[CLS_OF[s]] = float(sk[di, dj]) * float(np.sqrt(np.pi) / 2)
        _cached["key"] = key
        _cached["wsk"] = wsk_cls
        _cached["nc"] = _build(wsk_cls, gamma)
        _cached["lhst"] = _make_lhst(wsk_cls)
    return _cached["nc"], _cached["lhst"]


def kernel(x, spatial_kernel, sigma_color):
    x = np.ascontiguousarray(np.asarray(x, dtype=np.float32))
    sk = np.asarray(spatial_kernel, dtype=np.float64)
    sigma = float(np.asarray(sigma_color))
    gamma = 1.0 / (np.sqrt(2.0) * sigma)

    imgs = x.reshape(N_IMGS, H, W)
    xpad = np.pad(imgs, ((0, 0), (2, 2), (4, 4)), mode="reflect")
    halves_f32 = np.stack(
        [xpad[:, 0:PIECE_ROWS, :], xpad[:, HALF : HALF + PIECE_ROWS, :]], 1
    ).reshape(N_IMGS * 2, PIECE_ROWS, PIECE_COLS)
    halves = halves_f32.astype(np.float16)

    nc, lhst = _get_nc(sk, gamma)

    # core k even: halves [3k, 3k+1, 3k+2]; odd: [3k+1, 3k+2, 3k]
    # (pieces 0,1 always form a full-image chain; piece 2 is a lone chain)
    core_halves = []
    for k in range(N_CORES):
        if k % 2 == 0:
            core_halves.append([3 * k, 3 * k + 1, 3 * k + 2])
        else:
            core_halves.append([3 * k + 1, 3 * k + 2, 3 * k])

    in_maps = []
    for k in range(N_CORES):
        hs = core_halves[k]
        strips = np.stack(
            [_strip_planes(halves_f32[hs[0]], gamma),
             _strip_planes(halves_f32[hs[2]], gamma)]
        )
        in_maps.append({
            "x_in": np.ascontiguousarray(halves[hs]),
            "strips": np.ascontiguousarray(strips),
            "lhst": lhst,
        })

    trace = os.environ.get("BILATERAL_TRACE", "0") == "1"
    res = bass_utils.run_bass_kernel_spmd(
        nc, in_maps, core_ids=list(range(N_CORES)), trace=trace
    )
    kernel.last_results = res

    out = np.empty((N_IMGS * 2, HALF, W), dtype=np.float32)
    for k in range(N_CORES):
        for i, h in enumerate(core_halves[k]):
            out[h] = res.results[k]["y_out"][i]
    return (
        out.reshape(N_IMGS, 2, HALF, W)
        .reshape(N_IMGS, H, W)
        .reshape(B, C, H, W)
        .astype(np.float32)
    )


kernel.last_results = None


# revision 10
# speedup vs baseline: 1.0117x; 1.0117x over previous
"""Bilateral filter (5x5, reflect pad) on 8 Trainium2 NeuronCores.

Contract: kernel(**inputs) takes the FULL inputs
  x:              [4, 3, 512, 512] f32
  spatial_kernel: [5, 5] f32
  sigma_color:    scalar f32
and returns the FULL output [4, 3, 512, 512] f32.

v2: paired-plane algorithm. The bilateral weight between two pixels is
symmetric, so the 24 non-center taps form 12 point-symmetric pairs:
  w_refl[i,j] = w_rep[i-di', j-dj']      (di' = di-2, dj' = dj-2)
  u_refl[i,j] = -u_rep[i-di', j-dj']     (u = w * d, d = tap - center)
Only 12 "representative" planes (di in {3,4} x dj, plus (2,3),(2,4)) are
computed (halves ACT + DVE work vs all 24). Reductions
  S = 1 + sum_t wsk_t (w_t + w_t_shifted)
  U =     sum_t wsk_t (u_t - u_t_shifted)
run on the PE as 51 matmuls/tile: scaled identities for direct terms,
scaled shifted identities (superdiagonal) for row-shifted terms, and two
tiny seam matmuls for the 2 top rows whose shifted reads fall into the
previous tile (strips gathered by SBUF-SBUF DMA from the previous tile's
planes; chain-first tiles get host-computed strips). Output:
  out = center + U * reciprocal(S).

Sharding: each core gets one full image (512-row chain of 4 tiles) plus
one lone half image (256-row chain of 2 tiles) as three [260, 520] fp16
pieces (rows +-2, cols +-4 reflect halo), converted to fp16 on the host.
"""

import os

import numpy as np

import bass_rust
import concourse.bacc as bacc
import concourse.bass as bass
import concourse.mybir as mybir
import concourse.tile as tile
from concourse import bass_utils

F32 = mybir.dt.float32
FP16 = mybir.dt.float16
AF = mybir.ActivationFunctionType
ALU = mybir.AluOpType

N_CORES = 8
K = 5
B, C, H, W = 4, 3, 512, 512
N_IMGS = B * C                    # 12
HALF = 256
PIECE_ROWS = HALF + 4             # 260 (rows +-2)
PIECE_COLS = W + 8                # 520 (cols +-4)
PLANE_COLS = W + 4                # 516
NSLOT = 12

# plane slots: 0..4 = (3, 0..4), 5..9 = (4, 0..4), 10 = (2,3), 11 = (2,4)
REPS = [(3, d) for d in range(5)] + [(4, d) for d in range(5)] + [(2, 3), (2, 4)]
CLS_VALS = [1, 2, 4, 5, 8]        # distinct (di-2)^2 + (dj-2)^2 of reps
CLS_OF = [CLS_VALS.index((di - 2) ** 2 + (dj - 2) ** 2) for di, dj in REPS]

# lhsT pack layout (22 matrices of [128, 128] fp16)
LT_ONES = 0                       # row 0 = ones (center tap: S += 1)
LT_A = 1                          # +wsk_c * I          (5: cls 0..4)
LT_B = 6                          # -wsk_c * I, c in {1, 4} -> cls idx {0, 2}
LT_Z1 = 8                         # +wsk_c * Z1, c in {1,2,5} -> cls {0,1,3}
LT_Z2 = 11                        # +wsk_c * Z2, c in {4,5,8} -> cls {2,3,4}
LT_N1 = 14                        # -wsk_c * Z1
LT_N2 = 17                        # -wsk_c * Z2
LT_SEAM_S = 20
LT_SEAM_U = 21
N_MAT = 22
B_IDX = {0: 6, 2: 7}              # cls idx -> B slot
Z1_IDX = {0: 8, 1: 9, 3: 10}
Z2_IDX = {2: 11, 3: 12, 4: 13}
N1_IDX = {0: 14, 1: 15, 3: 16}
N2_IDX = {2: 17, 3: 18, 4: 19}

SUB_GROUPS = [(3, 0, 0, 5), (4, 0, 5, 5), (2, 3, 10, 2)]  # (di, dj0, slot0, g)

PLANE_EXT = NSLOT * PLANE_COLS    # 6192 elems per partition in W/U planes
NORM = float(2.0 / np.sqrt(np.pi))   # Derivative_Erf amplitude

_cached = {}


def _build(wsk_cls: dict, gamma: float) -> bass.Bass:
    """Per-core Bass module (SPMD: same NEFF on all 8 cores)."""
    nc = bacc.Bacc("TRN2", target_bir_lowering=False, debug=False)
    x_in = nc.dram_tensor(
        "x_in", [3, PIECE_ROWS, PIECE_COLS], FP16, kind="ExternalInput"
    ).ap()
    strips_in = nc.dram_tensor(
        "strips", [2, 2, 2, NSLOT, PLANE_COLS], FP16, kind="ExternalInput"
    ).ap()
    lhst_in = nc.dram_tensor(
        "lhst", [128, N_MAT, 128], FP16, kind="ExternalInput"
    ).ap()
    y_out = nc.dram_tensor(
        "y_out", [3, HALF, W], F32, kind="ExternalOutput"
    ).ap()

    with tile.TileContext(nc) as tc:
        with (
            tc.tile_pool(name="const_pool", bufs=1) as const_pool,
            tc.tile_pool(name="slab_pool", bufs=2) as slab_pool,
            tc.tile_pool(name="d_pool", bufs=2) as d_pool,
            tc.tile_pool(name="w_pool", bufs=2) as w_pool,
            tc.tile_pool(name="u_pool", bufs=2) as u_pool,
            tc.tile_pool(name="seam_pool", bufs=2) as seam_pool,
            tc.tile_pool(name="epi_pool", bufs=2) as epi_pool,
            tc.tile_pool(name="psum_pool", bufs=2, space="PSUM") as psum_pool,
        ):
            lhst = const_pool.tile([128, N_MAT, 128], FP16, tag="lhst",
                                   name="lhst")
            nc.sync.dma_start(lhst[:, :, :], lhst_in)
            ones_row = const_pool.tile([1, W], FP16, tag="ones", name="ones")
            nc.gpsimd.memset(ones_row[:, :], 1.0)

            # PE warmup: dummy matmuls depending only on the memset keep the
            # PE HAM busy through the first tile's plane production, so the
            # real burst starts at full clock (2.4 GHz, not cold 1.2).
            warm_ps = psum_pool.tile([128, W], F32, tag="warm", name="warm")
            for wi in range(0):
                nc.tensor.matmul(warm_ps[:, :], ones_row[0:1, 0:128],
                                 ones_row[0:1, :], start=(wi == 0),
                                 stop=(wi == 13))

            # tiles: (piece, r0, chain_first, chain_idx)
            tiles = [
                (0, 0, True, 0), (0, 128, False, 0),
                (1, 0, False, 0), (1, 128, False, 0),
                (2, 0, True, 1), (2, 128, False, 1),
            ]
            st = {}

            def prod(ti):
                """Plane production for tile ti: slab DMA, seam gathers,
                d -> w -> u, col-pair presums."""
                pc, r0, first, ch = tiles[ti]
                slab = slab_pool.tile([128, 3, PIECE_COLS], FP16, tag="slab",
                                      name=f"slab{ti}")
                src = x_in[pc, r0 + 2 : r0 + 2 + 128, :].copy()
                src.ap = bass_rust.VecI64Pair(
                    [(PIECE_COLS, 128), (PIECE_COLS, 3), (1, PIECE_COLS)]
                )
                nc.sync.dma_start(slab[:, :, :], src)
                slab_base = slab[:, :, :].offset

                seamW = seam_pool.tile([15, W], FP16, tag="sw", name=f"sw{ti}")
                seamU = seam_pool.tile([15, W], FP16, tag="su", name=f"su{ti}")
                for dst, which in ((seamW, 0), (seamU, 1)):
                    if first:
                        base = ch * (2 * 2 * PLANE_EXT) + which * (2 * PLANE_EXT)
                        proto = strips_in
                        off0 = base + 1 * PLANE_EXT + 4          # row ps=-1
                        off1 = base + 0 * PLANE_EXT + 5 * PLANE_COLS + 4
                    else:
                        prev = st[ti - 1]["W"] if which == 0 else st[ti - 1]["U"]
                        proto = prev[:, :, :]
                        pbase = proto.offset
                        off0 = pbase + 127 * PLANE_EXT + 4       # prev row 127
                        off1 = pbase + 126 * PLANE_EXT + 5 * PLANE_COLS + 4
                    v = proto.copy()
                    v.ap = bass_rust.VecI64Pair(
                        [(PLANE_EXT, 1), (PLANE_COLS - 1, 5), (1, W)]
                    )
                    v.offset = off0
                    nc.sync.dma_start(dst[0:5, :], v)
                    v2 = proto.copy()
                    v2.ap = bass_rust.VecI64Pair(
                        [(PLANE_EXT, 2), (PLANE_COLS - 1, 5), (1, W)]
                    )
                    v2.offset = off1
                    nc.sync.dma_start(dst[5:15, :], v2)

                d_all = d_pool.tile([128, NSLOT, PLANE_COLS], FP16, tag="d",
                                    name=f"d{ti}")
                W_all = w_pool.tile([128, NSLOT, PLANE_COLS], FP16, tag="w",
                                    name=f"w{ti}")
                U_all = u_pool.tile([128, NSLOT, PLANE_COLS], FP16, tag="u",
                                    name=f"u{ti}")
                for di, dj0, s0, g in SUB_GROUPS:
                    tap = slab[:, :, :].copy()
                    tap.ap = bass_rust.VecI64Pair(
                        [(3 * PIECE_COLS, 128), (1, g), (1, PLANE_COLS)]
                    )
                    tap.offset = slab_base + (di - 2) * PIECE_COLS + dj0
                    cen = slab[:, :, :].copy()
                    cen.ap = bass_rust.VecI64Pair(
                        [(3 * PIECE_COLS, 128), (0, g), (1, PLANE_COLS)]
                    )
                    cen.offset = slab_base + 2
                    nc.vector.tensor_sub(d_all[:, s0 : s0 + g, :], tap, cen)
                    nc.scalar.activation(W_all[:, s0 : s0 + g, :],
                                         d_all[:, s0 : s0 + g, :],
                                         AF.Derivative_Erf, scale=float(gamma))
                    nc.vector.tensor_mul(U_all[:, s0 : s0 + g, :],
                                         W_all[:, s0 : s0 + g, :],
                                         d_all[:, s0 : s0 + g, :])

                vt = seam_pool.tile([128, 2, W], FP16, tag="v", name=f"v{ti}")
                zt = seam_pool.tile([128, 2, W], FP16, tag="z", name=f"z{ti}")
                wsh = W_all[:, :, :].copy()
                wsh.ap = bass_rust.VecI64Pair(
                    [(PLANE_EXT, 128), (PLANE_COLS - 1, 2), (1, W)]
                )
                wsh.offset = W_all[:, :, :].offset + 10 * PLANE_COLS + 1
                nc.vector.tensor_add(vt[:, :, :], W_all[:, 10:12, 2 : 2 + W],
                                     wsh)
                ush = U_all[:, :, :].copy()
                ush.ap = bass_rust.VecI64Pair(
                    [(PLANE_EXT, 128), (PLANE_COLS - 1, 2), (1, W)]
                )
                ush.offset = U_all[:, :, :].offset + 10 * PLANE_COLS + 1
                nc.vector.tensor_sub(zt[:, :, :], U_all[:, 10:12, 2 : 2 + W],
                                     ush)
                st[ti] = dict(slab=slab, slab_base=slab_base, seamW=seamW,
                              seamU=seamU, W=W_all, U=U_all, vt=vt, zt=zt)

            def burst_epi(ti):
                pc, r0, first, ch = tiles[ti]
                s_ = st[ti]
                W_all, U_all = s_["W"], s_["U"]
                S_ps = psum_pool.tile([128, W], F32, tag="S", name=f"S{ti}")
                U_ps = psum_pool.tile([128, W], F32, tag="U", name=f"U{ti}")
                for s in range(10):
                    c = CLS_OF[s]
                    a = 4 - REPS[s][1]
                    sshift = Z1_IDX[c] if s < 5 else Z2_IDX[c]
                    ushift = N1_IDX[c] if s < 5 else N2_IDX[c]
                    nc.tensor.matmul(S_ps[:, :], lhst[:, LT_A + c, :],
                                     W_all[:, s, 2 : 2 + W],
                                     start=(s == 0), stop=False)
                    nc.tensor.matmul(U_ps[:, :], lhst[:, LT_A + c, :],
                                     U_all[:, s, 2 : 2 + W],
                                     start=(s == 0), stop=False)
                    nc.tensor.matmul(S_ps[:, :], lhst[:, sshift, :],
                                     W_all[:, s, a : a + W],
                                     start=False, stop=False)
                    nc.tensor.matmul(U_ps[:, :], lhst[:, ushift, :],
                                     U_all[:, s, a : a + W],
                                     start=False, stop=False)
                for i, s in enumerate((10, 11)):
                    c = CLS_OF[s]
                    nc.tensor.matmul(S_ps[:, :], lhst[:, LT_A + c, :],
                                     s_["vt"][:, i, :], start=False, stop=False)
                    nc.tensor.matmul(U_ps[:, :], lhst[:, LT_A + c, :],
                                     s_["zt"][:, i, :], start=False, stop=False)
                nc.tensor.matmul(S_ps[:, :], lhst[0:15, LT_SEAM_S, :],
                                 s_["seamW"][0:15, :], start=False, stop=False)
                nc.tensor.matmul(S_ps[:, :], lhst[0:1, LT_ONES, :],
                                 ones_row[0:1, :], start=False, stop=True)
                nc.tensor.matmul(U_ps[:, :], lhst[0:15, LT_SEAM_U, :],
                                 s_["seamU"][0:15, :], start=False, stop=True)

                R = epi_pool.tile([128, W], F32, tag="R", name=f"R{ti}")
                nc.vector.reciprocal_approx_fast(R[:, :], S_ps[:, :])
                UR = epi_pool.tile([128, W], F32, tag="UR", name=f"UR{ti}")
                nc.vector.scalar_tensor_tensor(UR[:, :], U_ps[:, :], 1.0,
                                               R[:, :], ALU.mult, ALU.mult)
                out_t = epi_pool.tile([128, W], F32, tag="out", name=f"o{ti}")
                cen = s_["slab"][:, :, :].copy()
                cen.ap = bass_rust.VecI64Pair([(3 * PIECE_COLS, 128), (1, W)])
                cen.offset = s_["slab_base"] + 4
                nc.vector.tensor_add(out_t[:, :], UR[:, :], cen)
                nc.sync.dma_start(y_out[pc, r0 : r0 + 128, :], out_t[:, :])

            # software-pipelined emission: production runs one tile ahead
            prod(0)
            for ti in range(6):
                if ti + 1 < 6:
                    prod(ti + 1)
                burst_epi(ti)
    nc.compile()
    return nc


def _make_lhst(wsk_cls: dict) -> np.ndarray:
    """[128, N_MAT, 128] fp16 lhsT pack."""
    mats = np.zeros((N_MAT, 128, 128), dtype=np.float32)
    eye = np.eye(128, dtype=np.float32)
    z1 = np.zeros((128, 128), dtype=np.float32)
    z1[np.arange(127), np.arange(1, 128)] = 1.0   # Z1[p, p+1] = 1
    z2 = np.zeros((128, 128), dtype=np.float32)
    z2[np.arange(126), np.arange(2, 128)] = 1.0
    mats[LT_ONES, 0, :] = 1.0
    for ci in range(5):
        mats[LT_A + ci] = wsk_cls[ci] * eye
    for ci, sl in B_IDX.items():
        mats[sl] = -wsk_cls[ci] * eye
    for ci, sl in Z1_IDX.items():
        mats[sl] = wsk_cls[ci] * z1
    for ci, sl in Z2_IDX.items():
        mats[sl] = wsk_cls[ci] * z2
    for ci, sl in N1_IDX.items():
        mats[sl] = -wsk_cls[ci] * z1
    for ci, sl in N2_IDX.items():
        mats[sl] = -wsk_cls[ci] * z2
    # seam lhsTs: rows 0..4 (dlt_i=1 pairs, slots 0..4) -> out row 0;
    # rows 5..9 (dlt_i=2, ps=-2) -> out row 0; rows 10..14 (ps=-1) -> row 1
    for r in range(5):
        mats[LT_SEAM_S, r, 0] = wsk_cls[CLS_OF[r]]
        mats[LT_SEAM_U, r, 0] = -wsk_cls[CLS_OF[r]]
        mats[LT_SEAM_S, 5 + r, 0] = wsk_cls[CLS_OF[5 + r]]
        mats[LT_SEAM_U, 5 + r, 0] = -wsk_cls[CLS_OF[5 + r]]
        mats[LT_SEAM_S, 10 + r, 1] = wsk_cls[CLS_OF[5 + r]]
        mats[LT_SEAM_U, 10 + r, 1] = -wsk_cls[CLS_OF[5 + r]]
    return np.ascontiguousarray(
        np.transpose(mats, (1, 0, 2)).astype(np.float16)
    )


def _strip_planes(piece: np.ndarray, gamma: float) -> np.ndarray:
    """Host chain-first strips: [2(w/u), 2(row: ps=-2,-1), 12, 516] fp16.
    piece: [260, 520] f32. Matches device plane values (incl. 2/sqrt(pi))."""
    out = np.zeros((2, 2, NSLOT, PLANE_COLS), dtype=np.float32)
    cols = np.arange(PLANE_COLS)
    for s, (di, dj) in enumerate(REPS):
        for ri, p in enumerate((-2, -1)):
            tap = piece[p + di, cols + dj]
            cen = piece[p + 2, cols + 2]
            d = (tap - cen).astype(np.float32)
            w = NORM * np.exp(-(gamma**2) * d * d)
            out[0, ri, s] = w
            out[1, ri, s] = w * d
    return out.astype(np.float16)


def _get_nc(sk: np.ndarray, gamma: float):
    key = (sk.tobytes(), float(gamma))
    if _cached.get("key") != key:
        wsk_cls = {}
        for s, (di, dj) in enumerate(REPS):
            # fold sqrt(pi)/2 so that wsk * D_ERF = sk * exp(-g^2 d^2)
            wsk_cls[CLS_OF[s]] = float(sk[di, dj]) * float(np.sqrt(np.pi) / 2)
        _cached["key"] = key
        _cached["wsk"] = wsk_cls
        _cached["nc"] = _build(wsk_cls, gamma)
        _cached["lhst"] = _make_lhst(wsk_cls)
    return _cached["nc"], _cached["lhst"]


def kernel(x, spatial_kernel, sigma_color):
    x = np.ascontiguousarray(np.asarray(x, dtype=np.float32))
    sk = np.asarray(spatial_kernel, dtype=np.float64)
    sigma = float(np.asarray(sigma_color))
    gamma = 1.0 / (np.sqrt(2.0) * sigma)

    imgs = x.reshape(N_IMGS, H, W)
    xpad = np.pad(imgs, ((0, 0), (2, 2), (4, 4)), mode="reflect")
    halves_f32 = np.stack(
        [xpad[:, 0:PIECE_ROWS, :], xpad[:, HALF : HALF + PIECE_ROWS, :]], 1
    ).reshape(N_IMGS * 2, PIECE_ROWS, PIECE_COLS)
    halves = halves_f32.astype(np.float16)

    nc, lhst = _get_nc(sk, gamma)

    # core k even: halves [3k, 3k+1, 3k+2]; odd: [3k+1, 3k+2, 3k]
    # (pieces 0,1 always form a full-image chain; piece 2 is a lone chain)
    core_halves = []
    for k in range(N_CORES):
        if k % 2 == 0:
            core_halves.append([3 * k, 3 * k + 1, 3 * k + 2])
        else:
            core_halves.append([3 * k + 1, 3 * k + 2, 3 * k])

    in_maps = []
    for k in range(N_CORES):
        hs = core_halves[k]
        strips = np.stack(
            [_strip_planes(halves_f32[hs[0]], gamma),
             _strip_planes(halves_f32[hs[2]], gamma)]
        )
        in_maps.append({
            "x_in": np.ascontiguousarray(halves[hs]),
            "strips": np.ascontiguousarray(strips),
            "lhst": lhst,
        })

    trace = os.environ.get("BILATERAL_TRACE", "0") == "1"
    res = bass_utils.run_bass_kernel_spmd(
        nc, in_maps, core_ids=list(range(N_CORES)), trace=trace
    )
    kernel.last_results = res

    out = np.empty((N_IMGS * 2, HALF, W), dtype=np.float32)
    for k in range(N_CORES):
        for i, h in enumerate(core_halves[k]):
            out[h] = res.results[k]["y_out"][i]
    return (
        out.reshape(N_IMGS, 2, HALF, W)
        .reshape(N_IMGS, H, W)
        .reshape(B, C, H, W)
        .astype(np.float32)
    )


kernel.last_results = None


# revision 11
# speedup vs baseline: 1.0205x; 1.0087x over previous
"""Bilateral filter (5x5, reflect pad) on 8 Trainium2 NeuronCores.

Contract: kernel(**inputs) takes the FULL inputs
  x:              [4, 3, 512, 512] f32
  spatial_kernel: [5, 5] f32
  sigma_color:    scalar f32
and returns the FULL output [4, 3, 512, 512] f32.

v2: paired-plane algorithm. The bilateral weight between two pixels is
symmetric, so the 24 non-center taps form 12 point-symmetric pairs:
  w_refl[i,j] = w_rep[i-di', j-dj']      (di' = di-2, dj' = dj-2)
  u_refl[i,j] = -u_rep[i-di', j-dj']     (u = w * d, d = tap - center)
Only 12 "representative" planes (di in {3,4} x dj, plus (2,3),(2,4)) are
computed (halves ACT + DVE work vs all 24). Reductions
  S = 1 + sum_t wsk_t (w_t + w_t_shifted)
  U =     sum_t wsk_t (u_t - u_t_shifted)
run on the PE as 51 matmuls/tile: scaled identities for direct terms,
scaled shifted identities (superdiagonal) for row-shifted terms, and two
tiny seam matmuls for the 2 top rows whose shifted reads fall into the
previous tile (strips gathered by SBUF-SBUF DMA from the previous tile's
planes; chain-first tiles get host-computed strips). Output:
  out = center + U * reciprocal(S).

Sharding: each core gets one full image (512-row chain of 4 tiles) plus
one lone half image (256-row chain of 2 tiles) as three [260, 520] fp16
pieces (rows +-2, cols +-4 reflect halo), converted to fp16 on the host.
"""

import os

import numpy as np

import bass_rust
import concourse.bacc as bacc
import concourse.bass as bass
import concourse.mybir as mybir
import concourse.tile as tile
from concourse import bass_utils

F32 = mybir.dt.float32
FP16 = mybir.dt.float16
AF = mybir.ActivationFunctionType
ALU = mybir.AluOpType

N_CORES = 8
K = 5
B, C, H, W = 4, 3, 512, 512
N_IMGS = B * C                    # 12
HALF = 256
PIECE_ROWS = HALF + 4             # 260 (rows +-2)
PIECE_COLS = W + 8                # 520 (cols +-4)
PLANE_COLS = W + 4                # 516
NSLOT = 12

# plane slots: 0..4 = (3, 0..4), 5..9 = (4, 0..4), 10 = (2,3), 11 = (2,4)
REPS = [(3, d) for d in range(5)] + [(4, d) for d in range(5)] + [(2, 3), (2, 4)]
CLS_VALS = [1, 2, 4, 5, 8]        # distinct (di-2)^2 + (dj-2)^2 of reps
CLS_OF = [CLS_VALS.index((di - 2) ** 2 + (dj - 2) ** 2) for di, dj in REPS]

# lhsT pack layout (22 matrices of [128, 128] fp16)
LT_ONES = 0                       # row 0 = ones (center tap: S += 1)
LT_A = 1                          # +wsk_c * I          (5: cls 0..4)
LT_B = 6                          # -wsk_c * I, c in {1, 4} -> cls idx {0, 2}
LT_Z1 = 8                         # +wsk_c * Z1, c in {1,2,5} -> cls {0,1,3}
LT_Z2 = 11                        # +wsk_c * Z2, c in {4,5,8} -> cls {2,3,4}
LT_N1 = 14                        # -wsk_c * Z1
LT_N2 = 17                        # -wsk_c * Z2
LT_SEAM_S = 20
LT_SEAM_U = 21
N_MAT = 22
B_IDX = {0: 6, 2: 7}              # cls idx -> B slot
Z1_IDX = {0: 8, 1: 9, 3: 10}
Z2_IDX = {2: 11, 3: 12, 4: 13}
N1_IDX = {0: 14, 1: 15, 3: 16}
N2_IDX = {2: 17, 3: 18, 4: 19}

SUB_GROUPS = [(3, 0, 0, 5), (4, 0, 5, 5), (2, 3, 10, 2)]  # (di, dj0, slot0, g)
USE_VZ = False

PLANE_EXT = NSLOT * PLANE_COLS    # 6192 elems per partition in W/U planes
NORM = float(2.0 / np.sqrt(np.pi))   # Derivative_Erf amplitude

_cached = {}


def _build(wsk_cls: dict, gamma: float) -> bass.Bass:
    """Per-core Bass module (SPMD: same NEFF on all 8 cores)."""
    nc = bacc.Bacc("TRN2", target_bir_lowering=False, debug=False)
    x_in = nc.dram_tensor(
        "x_in", [3, PIECE_ROWS, PIECE_COLS], FP16, kind="ExternalInput"
    ).ap()
    strips_in = nc.dram_tensor(
        "strips", [2, 2, 2, NSLOT, PLANE_COLS], FP16, kind="ExternalInput"
    ).ap()
    lhst_in = nc.dram_tensor(
        "lhst", [128, N_MAT, 128], FP16, kind="ExternalInput"
    ).ap()
    y_out = nc.dram_tensor(
        "y_out", [3, HALF, W], F32, kind="ExternalOutput"
    ).ap()

    with tile.TileContext(nc) as tc:
        with (
            tc.tile_pool(name="const_pool", bufs=1) as const_pool,
            tc.tile_pool(name="slab_pool", bufs=2) as slab_pool,
            tc.tile_pool(name="d_pool", bufs=2) as d_pool,
            tc.tile_pool(name="w_pool", bufs=2) as w_pool,
            tc.tile_pool(name="u_pool", bufs=2) as u_pool,
            tc.tile_pool(name="seam_pool", bufs=2) as seam_pool,
            tc.tile_pool(name="epi_pool", bufs=2) as epi_pool,
            tc.tile_pool(name="psum_pool", bufs=2, space="PSUM") as psum_pool,
        ):
            lhst = const_pool.tile([128, N_MAT, 128], FP16, tag="lhst",
                                   name="lhst")
            nc.sync.dma_start(lhst[:, :, :], lhst_in)
            ones_row = const_pool.tile([1, W], FP16, tag="ones", name="ones")
            nc.gpsimd.memset(ones_row[:, :], 1.0)

            # PE warmup: dummy matmuls depending only on the memset keep the
            # PE HAM busy through the first tile's plane production, so the
            # real burst starts at full clock (2.4 GHz, not cold 1.2).
            warm_ps = psum_pool.tile([128, W], F32, tag="warm", name="warm")
            for wi in range(0):
                nc.tensor.matmul(warm_ps[:, :], ones_row[0:1, 0:128],
                                 ones_row[0:1, :], start=(wi == 0),
                                 stop=(wi == 13))

            # tiles: (piece, r0, chain_first, chain_idx)
            tiles = [
                (0, 0, True, 0), (0, 128, False, 0),
                (1, 0, False, 0), (1, 128, False, 0),
                (2, 0, True, 1), (2, 128, False, 1),
            ]
            st = {}

            def prod(ti):
                """Plane production for tile ti: slab DMA, seam gathers,
                d -> w -> u, col-pair presums."""
                pc, r0, first, ch = tiles[ti]
                slab = slab_pool.tile([128, 3, PIECE_COLS], FP16, tag="slab",
                                      name=f"slab{ti}")
                src = x_in[pc, r0 + 2 : r0 + 2 + 128, :].copy()
                src.ap = bass_rust.VecI64Pair(
                    [(PIECE_COLS, 128), (PIECE_COLS, 3), (1, PIECE_COLS)]
                )
                nc.sync.dma_start(slab[:, :, :], src)
                slab_base = slab[:, :, :].offset

                seamW = seam_pool.tile([15, W], FP16, tag="sw", name=f"sw{ti}")
                seamU = seam_pool.tile([15, W], FP16, tag="su", name=f"su{ti}")
                for dst, which in ((seamW, 0), (seamU, 1)):
                    if first:
                        base = ch * (2 * 2 * PLANE_EXT) + which * (2 * PLANE_EXT)
                        proto = strips_in
                        off0 = base + 1 * PLANE_EXT + 4          # row ps=-1
                        off1 = base + 0 * PLANE_EXT + 5 * PLANE_COLS + 4
                    else:
                        prev = st[ti - 1]["W"] if which == 0 else st[ti - 1]["U"]
                        proto = prev[:, :, :]
                        pbase = proto.offset
                        off0 = pbase + 127 * PLANE_EXT + 4       # prev row 127
                        off1 = pbase + 126 * PLANE_EXT + 5 * PLANE_COLS + 4
                    v = proto.copy()
                    v.ap = bass_rust.VecI64Pair(
                        [(PLANE_EXT, 1), (PLANE_COLS - 1, 5), (1, W)]
                    )
                    v.offset = off0
                    nc.sync.dma_start(dst[0:5, :], v)
                    v2 = proto.copy()
                    v2.ap = bass_rust.VecI64Pair(
                        [(PLANE_EXT, 2), (PLANE_COLS - 1, 5), (1, W)]
                    )
                    v2.offset = off1
                    nc.sync.dma_start(dst[5:15, :], v2)

                d_all = d_pool.tile([128, NSLOT, PLANE_COLS], FP16, tag="d",
                                    name=f"d{ti}")
                W_all = w_pool.tile([128, NSLOT, PLANE_COLS], FP16, tag="w",
                                    name=f"w{ti}")
                U_all = u_pool.tile([128, NSLOT, PLANE_COLS], FP16, tag="u",
                                    name=f"u{ti}")
                for di, dj0, s0, g in SUB_GROUPS:
                    tap = slab[:, :, :].copy()
                    tap.ap = bass_rust.VecI64Pair(
                        [(3 * PIECE_COLS, 128), (1, g), (1, PLANE_COLS)]
                    )
                    tap.offset = slab_base + (di - 2) * PIECE_COLS + dj0
                    cen = slab[:, :, :].copy()
                    cen.ap = bass_rust.VecI64Pair(
                        [(3 * PIECE_COLS, 128), (0, g), (1, PLANE_COLS)]
                    )
                    cen.offset = slab_base + 2
                    nc.vector.tensor_sub(d_all[:, s0 : s0 + g, :], tap, cen)
                    nc.scalar.activation(W_all[:, s0 : s0 + g, :],
                                         d_all[:, s0 : s0 + g, :],
                                         AF.Derivative_Erf, scale=float(gamma))
                    nc.vector.tensor_mul(U_all[:, s0 : s0 + g, :],
                                         W_all[:, s0 : s0 + g, :],
                                         d_all[:, s0 : s0 + g, :])

                vt = zt = None
                if not USE_VZ:
                    st[ti] = dict(slab=slab, slab_base=slab_base, seamW=seamW,
                                  seamU=seamU, W=W_all, U=U_all, vt=vt, zt=zt)
                    return
                vt = seam_pool.tile([128, 2, W], FP16, tag="v", name=f"v{ti}")
                zt = seam_pool.tile([128, 2, W], FP16, tag="z", name=f"z{ti}")
                wsh = W_all[:, :, :].copy()
                wsh.ap = bass_rust.VecI64Pair(
                    [(PLANE_EXT, 128), (PLANE_COLS - 1, 2), (1, W)]
                )
                wsh.offset = W_all[:, :, :].offset + 10 * PLANE_COLS + 1
                nc.vector.tensor_add(vt[:, :, :], W_all[:, 10:12, 2 : 2 + W],
                                     wsh)
                ush = U_all[:, :, :].copy()
                ush.ap = bass_rust.VecI64Pair(
                    [(PLANE_EXT, 128), (PLANE_COLS - 1, 2), (1, W)]
                )
                ush.offset = U_all[:, :, :].offset + 10 * PLANE_COLS + 1
                nc.vector.tensor_sub(zt[:, :, :], U_all[:, 10:12, 2 : 2 + W],
                                     ush)
                st[ti] = dict(slab=slab, slab_base=slab_base, seamW=seamW,
                              seamU=seamU, W=W_all, U=U_all, vt=vt, zt=zt)

            def burst_epi(ti):
                pc, r0, first, ch = tiles[ti]
                s_ = st[ti]
                W_all, U_all = s_["W"], s_["U"]
                S_ps = psum_pool.tile([128, W], F32, tag="S", name=f"S{ti}")
                U_ps = psum_pool.tile([128, W], F32, tag="U", name=f"U{ti}")
                for s in range(10):
                    c = CLS_OF[s]
                    a = 4 - REPS[s][1]
                    sshift = Z1_IDX[c] if s < 5 else Z2_IDX[c]
                    ushift = N1_IDX[c] if s < 5 else N2_IDX[c]
                    nc.tensor.matmul(S_ps[:, :], lhst[:, LT_A + c, :],
                                     W_all[:, s, 2 : 2 + W],
                                     start=(s == 0), stop=False)
                    nc.tensor.matmul(U_ps[:, :], lhst[:, LT_A + c, :],
                                     U_all[:, s, 2 : 2 + W],
                                     start=(s == 0), stop=False)
                    nc.tensor.matmul(S_ps[:, :], lhst[:, sshift, :],
                                     W_all[:, s, a : a + W],
                                     start=False, stop=False)
                    nc.tensor.matmul(U_ps[:, :], lhst[:, ushift, :],
                                     U_all[:, s, a : a + W],
                                     start=False, stop=False)
                if USE_VZ:
                    for i, s in enumerate((10, 11)):
                        c = CLS_OF[s]
                        nc.tensor.matmul(S_ps[:, :], lhst[:, LT_A + c, :],
                                         s_["vt"][:, i, :], start=False,
                                         stop=False)
                        nc.tensor.matmul(U_ps[:, :], lhst[:, LT_A + c, :],
                                         s_["zt"][:, i, :], start=False,
                                         stop=False)
                else:
                    for s in (10, 11):
                        c = CLS_OF[s]
                        a = 4 - REPS[s][1]
                        nc.tensor.matmul(S_ps[:, :], lhst[:, LT_A + c, :],
                                         W_all[:, s, 2 : 2 + W],
                                         start=False, stop=False)
                        nc.tensor.matmul(U_ps[:, :], lhst[:, LT_A + c, :],
                                         U_all[:, s, 2 : 2 + W],
                                         start=False, stop=False)
                        nc.tensor.matmul(S_ps[:, :], lhst[:, LT_A + c, :],
                                         W_all[:, s, a : a + W],
                                         start=False, stop=False)
                        nc.tensor.matmul(U_ps[:, :], lhst[:, B_IDX[c], :],
                                         U_all[:, s, a : a + W],
                                         start=False, stop=False)
                nc.tensor.matmul(S_ps[:, :], lhst[0:15, LT_SEAM_S, :],
                                 s_["seamW"][0:15, :], start=False, stop=False)
                nc.tensor.matmul(S_ps[:, :], lhst[0:1, LT_ONES, :],
                                 ones_row[0:1, :], start=False, stop=True)
                nc.tensor.matmul(U_ps[:, :], lhst[0:15, LT_SEAM_U, :],
                                 s_["seamU"][0:15, :], start=False, stop=True)

                R = epi_pool.tile([128, W], F32, tag="R", name=f"R{ti}")
                nc.vector.reciprocal_approx_fast(R[:, :], S_ps[:, :])
                UR = epi_pool.tile([128, W], F32, tag="UR", name=f"UR{ti}")
                nc.vector.scalar_tensor_tensor(UR[:, :], U_ps[:, :], 1.0,
                                               R[:, :], ALU.mult, ALU.mult)
                out_t = epi_pool.tile([128, W], F32, tag="out", name=f"o{ti}")
                cen = s_["slab"][:, :, :].copy()
                cen.ap = bass_rust.VecI64Pair([(3 * PIECE_COLS, 128), (1, W)])
                cen.offset = s_["slab_base"] + 4
                nc.vector.tensor_add(out_t[:, :], UR[:, :], cen)
                nc.sync.dma_start(y_out[pc, r0 : r0 + 128, :], out_t[:, :])

            # software-pipelined emission: production runs one tile ahead
            prod(0)
            for ti in range(6):
                if ti + 1 < 6:
                    prod(ti + 1)
                burst_epi(ti)
    nc.compile()
    return nc


def _make_lhst(wsk_cls: dict) -> np.ndarray:
    """[128, N_MAT, 128] fp16 lhsT pack."""
    mats = np.zeros((N_MAT, 128, 128), dtype=np.float32)
    eye = np.eye(128, dtype=np.float32)
    z1 = np.zeros((128, 128), dtype=np.float32)
    z1[np.arange(127), np.arange(1, 128)] = 1.0   # Z1[p, p+1] = 1
    z2 = np.zeros((128, 128), dtype=np.float32)
    z2[np.arange(126), np.arange(2, 128)] = 1.0
    mats[LT_ONES, 0, :] = 1.0
    for ci in range(5):
        mats[LT_A + ci] = wsk_cls[ci] * eye
    for ci, sl in B_IDX.items():
        mats[sl] = -wsk_cls[ci] * eye
    for ci, sl in Z1_IDX.items():
        mats[sl] = wsk_cls[ci] * z1
    for ci, sl in Z2_IDX.items():
        mats[sl] = wsk_cls[ci] * z2
    for ci, sl in N1_IDX.items():
        mats[sl] = -wsk_cls[ci] * z1
    for ci, sl in N2_IDX.items():
        mats[sl] = -wsk_cls[ci] * z2
    # seam lhsTs: rows 0..4 (dlt_i=1 pairs, slots 0..4) -> out row 0;
    # rows 5..9 (dlt_i=2, ps=-2) -> out row 0; rows 10..14 (ps=-1) -> row 1
    for r in range(5):
        mats[LT_SEAM_S, r, 0] = wsk_cls[CLS_OF[r]]
        mats[LT_SEAM_U, r, 0] = -wsk_cls[CLS_OF[r]]
        mats[LT_SEAM_S, 5 + r, 0] = wsk_cls[CLS_OF[5 + r]]
        mats[LT_SEAM_U, 5 + r, 0] = -wsk_cls[CLS_OF[5 + r]]
        mats[LT_SEAM_S, 10 + r, 1] = wsk_cls[CLS_OF[5 + r]]
        mats[LT_SEAM_U, 10 + r, 1] = -wsk_cls[CLS_OF[5 + r]]
    return np.ascontiguousarray(
        np.transpose(mats, (1, 0, 2)).astype(np.float16)
    )


def _strip_planes(piece: np.ndarray, gamma: float) -> np.ndarray:
    """Host chain-first strips: [2(w/u), 2(row: ps=-2,-1), 12, 516] fp16.
    piece: [260, 520] f32. Matches device plane values (incl. 2/sqrt(pi))."""
    out = np.zeros((2, 2, NSLOT, PLANE_COLS), dtype=np.float32)
    cols = np.arange(PLANE_COLS)
    for s, (di, dj) in enumerate(REPS):
        for ri, p in enumerate((-2, -1)):
            tap = piece[p + di, cols + dj]
            cen = piece[p + 2, cols + 2]
            d = (tap - cen).astype(np.float32)
            w = NORM * np.exp(-(gamma**2) * d * d)
            out[0, ri, s] = w
            out[1, ri, s] = w * d
    return out.astype(np.float16)


def _get_nc(sk: np.ndarray, gamma: float):
    key = (sk.tobytes(), float(gamma))
    if _cached.get("key") != key:
        wsk_cls = {}
        for s, (di, dj) in enumerate(REPS):
            # fold sqrt(pi)/2 so that wsk * D_ERF = sk * exp(-g^2 d^2)
            wsk_cls[CLS_OF[s]] = float(sk[di, dj]) * float(np.sqrt(np.pi) / 2)
        _cached["key"] = key
        _cached["wsk"] = wsk_cls
        _cached["nc"] = _build(wsk_cls, gamma)
        _cached["lhst"] = _make_lhst(wsk_cls)
    return _cached["nc"], _cached["lhst"]


def kernel(x, spatial_kernel, sigma_color):
    x = np.ascontiguousarray(np.asarray(x, dtype=np.float32))
    sk = np.asarray(spatial_kernel, dtype=np.float64)
    sigma = float(np.asarray(sigma_color))
    gamma = 1.0 / (np.sqrt(2.0) * sigma)

    imgs = x.reshape(N_IMGS, H, W)
    xpad = np.pad(imgs, ((0, 0), (2, 2), (4, 4)), mode="reflect")
    halves_f32 = np.stack(
        [xpad[:, 0:PIECE_ROWS, :], xpad[:, HALF : HALF + PIECE_ROWS, :]], 1
    ).reshape(N_IMGS * 2, PIECE_ROWS, PIECE_COLS)
    halves = halves_f32.astype(np.float16)

    nc, lhst = _get_nc(sk, gamma)

    # core k even: halves [3k, 3k+1, 3k+2]; odd: [3k+1, 3k+2, 3k]
    # (pieces 0,1 always form a full-image chain; piece 2 is a lone chain)
    core_halves = []
    for k in range(N_CORES):
        if k % 2 == 0:
            core_halves.append([3 * k, 3 * k + 1, 3 * k + 2])
        else:
            core_halves.append([3 * k + 1, 3 * k + 2, 3 * k])

    in_maps = []
    for k in range(N_CORES):
        hs = core_halves[k]
        strips = np.stack(
            [_strip_planes(halves_f32[hs[0]], gamma),
             _strip_planes(halves_f32[hs[2]], gamma)]
        )
        in_maps.append({
            "x_in": np.ascontiguousarray(halves[hs]),
            "strips": np.ascontiguousarray(strips),
            "lhst": lhst,
        })

    trace = os.environ.get("BILATERAL_TRACE", "0") == "1"
    res = bass_utils.run_bass_kernel_spmd(
        nc, in_maps, core_ids=list(range(N_CORES)), trace=trace
    )
    kernel.last_results = res

    out = np.empty((N_IMGS * 2, HALF, W), dtype=np.float32)
    for k in range(N_CORES):
        for i, h in enumerate(core_halves[k]):
            out[h] = res.results[k]["y_out"][i]
    return (
        out.reshape(N_IMGS, 2, HALF, W)
        .reshape(N_IMGS, H, W)
        .reshape(B, C, H, W)
        .astype(np.float32)
    )


kernel.last_results = None


# revision 12
# speedup vs baseline: 1.0705x; 1.0490x over previous
"""Bilateral filter (5x5, reflect pad) on 8 Trainium2 NeuronCores.

Contract: kernel(**inputs) takes the FULL inputs
  x:              [4, 3, 512, 512] f32
  spatial_kernel: [5, 5] f32
  sigma_color:    scalar f32
and returns the FULL output [4, 3, 512, 512] f32.

v2: paired-plane algorithm. The bilateral weight between two pixels is
symmetric, so the 24 non-center taps form 12 point-symmetric pairs:
  w_refl[i,j] = w_rep[i-di', j-dj']      (di' = di-2, dj' = dj-2)
  u_refl[i,j] = -u_rep[i-di', j-dj']     (u = w * d, d = tap - center)
Only 12 "representative" planes (di in {3,4} x dj, plus (2,3),(2,4)) are
computed (halves ACT + DVE work vs all 24). Reductions
  S = 1 + sum_t wsk_t (w_t + w_t_shifted)
  U =     sum_t wsk_t (u_t - u_t_shifted)
run on the PE as 51 matmuls/tile: scaled identities for direct terms,
scaled shifted identities (superdiagonal) for row-shifted terms, and two
tiny seam matmuls for the 2 top rows whose shifted reads fall into the
previous tile (strips gathered by SBUF-SBUF DMA from the previous tile's
planes; chain-first tiles get host-computed strips). Output:
  out = center + U * reciprocal(S).

Sharding: each core gets one full image (512-row chain of 4 tiles) plus
one lone half image (256-row chain of 2 tiles) as three [260, 520] fp16
pieces (rows +-2, cols +-4 reflect halo), converted to fp16 on the host.
"""

import os

import numpy as np

import bass_rust
import concourse.bacc as bacc
import concourse.bass as bass
import concourse.mybir as mybir
import concourse.tile as tile
from concourse import bass_utils

F32 = mybir.dt.float32
FP16 = mybir.dt.float16
AF = mybir.ActivationFunctionType
ALU = mybir.AluOpType

N_CORES = 8
K = 5
B, C, H, W = 4, 3, 512, 512
N_IMGS = B * C                    # 12
HALF = 256
PIECE_ROWS = HALF + 4             # 260 (rows +-2)
PIECE_COLS = W + 8                # 520 (cols +-4)
PLANE_COLS = W + 4                # 516
NSLOT = 12

# plane slots: 0..4 = (3, 0..4), 5..9 = (4, 0..4), 10 = (2,3), 11 = (2,4)
REPS = [(3, d) for d in range(5)] + [(4, d) for d in range(5)] + [(2, 3), (2, 4)]
CLS_VALS = [1, 2, 4, 5, 8]        # distinct (di-2)^2 + (dj-2)^2 of reps
CLS_OF = [CLS_VALS.index((di - 2) ** 2 + (dj - 2) ** 2) for di, dj in REPS]

# lhsT pack layout (22 matrices of [128, 128] fp16)
LT_ONES = 0                       # row 0 = ones (center tap: S += 1)
LT_A = 1                          # +wsk_c * I          (5: cls 0..4)
LT_B = 6                          # -wsk_c * I, c in {1, 4} -> cls idx {0, 2}
LT_Z1 = 8                         # +wsk_c * Z1, c in {1,2,5} -> cls {0,1,3}
LT_Z2 = 11                        # +wsk_c * Z2, c in {4,5,8} -> cls {2,3,4}
LT_N1 = 14                        # -wsk_c * Z1
LT_N2 = 17                        # -wsk_c * Z2
LT_SEAM_S = 20
LT_SEAM_U = 21
N_MAT = 22
B_IDX = {0: 6, 2: 7}              # cls idx -> B slot
Z1_IDX = {0: 8, 1: 9, 3: 10}
Z2_IDX = {2: 11, 3: 12, 4: 13}
N1_IDX = {0: 14, 1: 15, 3: 16}
N2_IDX = {2: 17, 3: 18, 4: 19}

SUB_GROUPS = [(3, 0, 0, 5), (4, 0, 5, 5), (2, 3, 10, 2)]  # (di, dj0, slot0, g)
USE_VZ = False

PLANE_EXT = NSLOT * PLANE_COLS    # 6192 elems per partition in W/U planes
NORM = float(2.0 / np.sqrt(np.pi))   # Derivative_Erf amplitude

_cached = {}


def _build(wsk_cls: dict, gamma: float) -> bass.Bass:
    """Per-core Bass module (SPMD: same NEFF on all 8 cores)."""
    nc = bacc.Bacc("TRN2", target_bir_lowering=False, debug=False)
    x_in = nc.dram_tensor(
        "x_in", [3, PIECE_ROWS, PIECE_COLS], FP16, kind="ExternalInput"
    ).ap()
    strips_in = nc.dram_tensor(
        "strips", [2, 2, 2, NSLOT, PLANE_COLS], FP16, kind="ExternalInput"
    ).ap()
    lhst_in = nc.dram_tensor(
        "lhst", [128, N_MAT, 128], FP16, kind="ExternalInput"
    ).ap()
    y_out = nc.dram_tensor(
        "y_out", [3, HALF, W], F32, kind="ExternalOutput"
    ).ap()

    with tile.TileContext(nc) as tc:
        with (
            tc.tile_pool(name="const_pool", bufs=1) as const_pool,
            tc.tile_pool(name="slab_pool", bufs=2) as slab_pool,
            tc.tile_pool(name="d_pool", bufs=2) as d_pool,
            tc.tile_pool(name="w_pool", bufs=2) as w_pool,
            tc.tile_pool(name="u_pool", bufs=2) as u_pool,
            tc.tile_pool(name="seam_pool", bufs=2) as seam_pool,
            tc.tile_pool(name="epi_pool", bufs=2) as epi_pool,
            tc.tile_pool(name="psum_pool", bufs=2, space="PSUM") as psum_pool,
        ):
            lhst = const_pool.tile([128, N_MAT, 128], FP16, tag="lhst",
                                   name="lhst")
            nc.sync.dma_start(lhst[:, :, :], lhst_in)
            ones_row = const_pool.tile([1, W], FP16, tag="ones", name="ones")
            nc.gpsimd.memset(ones_row[:, :], 1.0)

            # PE warmup: dummy matmuls depending only on the memset keep the
            # PE HAM busy through the first tile's plane production, so the
            # real burst starts at full clock (2.4 GHz, not cold 1.2).
            warm_ps = psum_pool.tile([128, W], F32, tag="warm", name="warm")
            for wi in range(0):
                nc.tensor.matmul(warm_ps[:, :], ones_row[0:1, 0:128],
                                 ones_row[0:1, :], start=(wi == 0),
                                 stop=(wi == 13))

            # tiles: (piece, r0, chain_first, chain_idx)
            tiles = [
                (0, 0, True, 0), (0, 128, False, 0),
                (1, 0, False, 0), (1, 128, False, 0),
                (2, 0, True, 1), (2, 128, False, 1),
            ]
            st = {}

            def prod(ti):
                """Plane production for tile ti: slab DMA, seam gathers,
                d -> w -> u, col-pair presums."""
                pc, r0, first, ch = tiles[ti]
                slab = slab_pool.tile([128, 3, PIECE_COLS], FP16, tag="slab",
                                      name=f"slab{ti}")
                src = x_in[pc, r0 + 2 : r0 + 2 + 128, :].copy()
                src.ap = bass_rust.VecI64Pair(
                    [(PIECE_COLS, 128), (PIECE_COLS, 3), (1, PIECE_COLS)]
                )
                nc.sync.dma_start(slab[:, :, :], src)
                slab_base = slab[:, :, :].offset

                seamW = seam_pool.tile([15, W], FP16, tag="sw", name=f"sw{ti}")
                seamU = seam_pool.tile([15, W], FP16, tag="su", name=f"su{ti}")
                for dst, which in ((seamW, 0), (seamU, 1)):
                    if first:
                        base = ch * (2 * 2 * PLANE_EXT) + which * (2 * PLANE_EXT)
                        proto = strips_in
                        off0 = base + 1 * PLANE_EXT + 4          # row ps=-1
                        off1 = base + 0 * PLANE_EXT + 5 * PLANE_COLS + 4
                    else:
                        prev = st[ti - 1]["W"] if which == 0 else st[ti - 1]["U"]
                        proto = prev[:, :, :]
                        pbase = proto.offset
                        off0 = pbase + 127 * PLANE_EXT + 4       # prev row 127
                        off1 = pbase + 126 * PLANE_EXT + 5 * PLANE_COLS + 4
                    v = proto.copy()
                    v.ap = bass_rust.VecI64Pair(
                        [(PLANE_EXT, 1), (PLANE_COLS - 1, 5), (1, W)]
                    )
                    v.offset = off0
                    nc.sync.dma_start(dst[0:5, :], v)
                    v2 = proto.copy()
                    v2.ap = bass_rust.VecI64Pair(
                        [(PLANE_EXT, 2), (PLANE_COLS - 1, 5), (1, W)]
                    )
                    v2.offset = off1
                    nc.sync.dma_start(dst[5:15, :], v2)

                d_all = d_pool.tile([128, NSLOT, PLANE_COLS], FP16, tag="d",
                                    name=f"d{ti}")
                W_all = w_pool.tile([128, NSLOT, PLANE_COLS], FP16, tag="w",
                                    name=f"w{ti}")
                U_all = u_pool.tile([128, NSLOT, PLANE_COLS], FP16, tag="u",
                                    name=f"u{ti}")
                for di, dj0, s0, g in SUB_GROUPS:
                    tap = slab[:, :, :].copy()
                    tap.ap = bass_rust.VecI64Pair(
                        [(3 * PIECE_COLS, 128), (1, g), (1, PLANE_COLS)]
                    )
                    tap.offset = slab_base + (di - 2) * PIECE_COLS + dj0
                    cen = slab[:, :, :].copy()
                    cen.ap = bass_rust.VecI64Pair(
                        [(3 * PIECE_COLS, 128), (0, g), (1, PLANE_COLS)]
                    )
                    cen.offset = slab_base + 2
                    nc.vector.tensor_sub(d_all[:, s0 : s0 + g, :], tap, cen)
                    nc.scalar.activation(W_all[:, s0 : s0 + g, :],
                                         d_all[:, s0 : s0 + g, :],
                                         AF.Derivative_Erf, scale=float(gamma))
                    nc.vector.tensor_mul(U_all[:, s0 : s0 + g, :],
                                         W_all[:, s0 : s0 + g, :],
                                         d_all[:, s0 : s0 + g, :])

                vt = zt = None
                if not USE_VZ:
                    st[ti] = dict(slab=slab, slab_base=slab_base, seamW=seamW,
                                  seamU=seamU, W=W_all, U=U_all, vt=vt, zt=zt)
                    return
                vt = seam_pool.tile([128, 2, W], FP16, tag="v", name=f"v{ti}")
                zt = seam_pool.tile([128, 2, W], FP16, tag="z", name=f"z{ti}")
                wsh = W_all[:, :, :].copy()
                wsh.ap = bass_rust.VecI64Pair(
                    [(PLANE_EXT, 128), (PLANE_COLS - 1, 2), (1, W)]
                )
                wsh.offset = W_all[:, :, :].offset + 10 * PLANE_COLS + 1
                nc.vector.tensor_add(vt[:, :, :], W_all[:, 10:12, 2 : 2 + W],
                                     wsh)
                ush = U_all[:, :, :].copy()
                ush.ap = bass_rust.VecI64Pair(
                    [(PLANE_EXT, 128), (PLANE_COLS - 1, 2), (1, W)]
                )
                ush.offset = U_all[:, :, :].offset + 10 * PLANE_COLS + 1
                nc.vector.tensor_sub(zt[:, :, :], U_all[:, 10:12, 2 : 2 + W],
                                     ush)
                st[ti] = dict(slab=slab, slab_base=slab_base, seamW=seamW,
                              seamU=seamU, W=W_all, U=U_all, vt=vt, zt=zt)

            def burst_epi(ti):
                pc, r0, first, ch = tiles[ti]
                s_ = st[ti]
                W_all, U_all = s_["W"], s_["U"]
                S_ps = psum_pool.tile([128, W], F32, tag="S", name=f"S{ti}")
                U_ps = psum_pool.tile([128, W], F32, tag="U", name=f"U{ti}")
                nc.tensor.matmul(S_ps[:, :], lhst[0:15, LT_SEAM_S, :],
                                 s_["seamW"][0:15, :], start=True, stop=False)
                nc.tensor.matmul(S_ps[:, :], lhst[0:1, LT_ONES, :],
                                 ones_row[0:1, :], start=False, stop=False)
                nc.tensor.matmul(U_ps[:, :], lhst[0:15, LT_SEAM_U, :],
                                 s_["seamU"][0:15, :], start=True, stop=False)
                for s in range(10):
                    c = CLS_OF[s]
                    a = 4 - REPS[s][1]
                    sshift = Z1_IDX[c] if s < 5 else Z2_IDX[c]
                    ushift = N1_IDX[c] if s < 5 else N2_IDX[c]
                    nc.tensor.matmul(S_ps[:, :], lhst[:, LT_A + c, :],
                                     W_all[:, s, 2 : 2 + W],
                                     start=False, stop=False)
                    nc.tensor.matmul(U_ps[:, :], lhst[:, LT_A + c, :],
                                     U_all[:, s, 2 : 2 + W],
                                     start=False, stop=False)
                    nc.tensor.matmul(S_ps[:, :], lhst[:, sshift, :],
                                     W_all[:, s, a : a + W],
                                     start=False, stop=False)
                    nc.tensor.matmul(U_ps[:, :], lhst[:, ushift, :],
                                     U_all[:, s, a : a + W],
                                     start=False, stop=False)
                if USE_VZ:
                    for i, s in enumerate((10, 11)):
                        c = CLS_OF[s]
                        nc.tensor.matmul(S_ps[:, :], lhst[:, LT_A + c, :],
                                         s_["vt"][:, i, :], start=False,
                                         stop=(s == 11))
                        nc.tensor.matmul(U_ps[:, :], lhst[:, LT_A + c, :],
                                         s_["zt"][:, i, :], start=False,
                                         stop=(s == 11))
                else:
                    for s in (10, 11):
                        c = CLS_OF[s]
                        a = 4 - REPS[s][1]
                        nc.tensor.matmul(S_ps[:, :], lhst[:, LT_A + c, :],
                                         W_all[:, s, 2 : 2 + W],
                                         start=False, stop=False)
                        nc.tensor.matmul(U_ps[:, :], lhst[:, LT_A + c, :],
                                         U_all[:, s, 2 : 2 + W],
                                         start=False, stop=False)
                        nc.tensor.matmul(S_ps[:, :], lhst[:, LT_A + c, :],
                                         W_all[:, s, a : a + W],
                                         start=False, stop=(s == 11))
                        nc.tensor.matmul(U_ps[:, :], lhst[:, B_IDX[c], :],
                                         U_all[:, s, a : a + W],
                                         start=False, stop=(s == 11))

                R = epi_pool.tile([128, W], F32, tag="R", name=f"R{ti}")
                nc.vector.reciprocal_approx_fast(R[:, :], S_ps[:, :])
                UR = epi_pool.tile([128, W], F32, tag="UR", name=f"UR{ti}")
                nc.vector.scalar_tensor_tensor(UR[:, :], U_ps[:, :], 1.0,
                                               R[:, :], ALU.mult, ALU.mult)
                out_t = epi_pool.tile([128, W], F32, tag="out", name=f"o{ti}")
                cen = s_["slab"][:, :, :].copy()
                cen.ap = bass_rust.VecI64Pair([(3 * PIECE_COLS, 128), (1, W)])
                cen.offset = s_["slab_base"] + 4
                nc.vector.tensor_add(out_t[:, :], UR[:, :], cen)
                nc.sync.dma_start(y_out[pc, r0 : r0 + 128, :], out_t[:, :])

            # software-pipelined emission: production runs one tile ahead
            prod(0)
            for ti in range(6):
                if ti + 1 < 6:
                    prod(ti + 1)
                burst_epi(ti)
    nc.compile()
    return nc


def _make_lhst(wsk_cls: dict) -> np.ndarray:
    """[128, N_MAT, 128] fp16 lhsT pack."""
    mats = np.zeros((N_MAT, 128, 128), dtype=np.float32)
    eye = np.eye(128, dtype=np.float32)
    z1 = np.zeros((128, 128), dtype=np.float32)
    z1[np.arange(127), np.arange(1, 128)] = 1.0   # Z1[p, p+1] = 1
    z2 = np.zeros((128, 128), dtype=np.float32)
    z2[np.arange(126), np.arange(2, 128)] = 1.0
    mats[LT_ONES, 0, :] = 1.0
    for ci in range(5):
        mats[LT_A + ci] = wsk_cls[ci] * eye
    for ci, sl in B_IDX.items():
        mats[sl] = -wsk_cls[ci] * eye
    for ci, sl in Z1_IDX.items():
        mats[sl] = wsk_cls[ci] * z1
    for ci, sl in Z2_IDX.items():
        mats[sl] = wsk_cls[ci] * z2
    for ci, sl in N1_IDX.items():
        mats[sl] = -wsk_cls[ci] * z1
    for ci, sl in N2_IDX.items():
        mats[sl] = -wsk_cls[ci] * z2
    # seam lhsTs: rows 0..4 (dlt_i=1 pairs, slots 0..4) -> out row 0;
    # rows 5..9 (dlt_i=2, ps=-2) -> out row 0; rows 10..14 (ps=-1) -> row 1
    for r in range(5):
        mats[LT_SEAM_S, r, 0] = wsk_cls[CLS_OF[r]]
        mats[LT_SEAM_U, r, 0] = -wsk_cls[CLS_OF[r]]
        mats[LT_SEAM_S, 5 + r, 0] = wsk_cls[CLS_OF[5 + r]]
        mats[LT_SEAM_U, 5 + r, 0] = -wsk_cls[CLS_OF[5 + r]]
        mats[LT_SEAM_S, 10 + r, 1] = wsk_cls[CLS_OF[5 + r]]
        mats[LT_SEAM_U, 10 + r, 1] = -wsk_cls[CLS_OF[5 + r]]
    return np.ascontiguousarray(
        np.transpose(mats, (1, 0, 2)).astype(np.float16)
    )


def _strip_planes(piece: np.ndarray, gamma: float) -> np.ndarray:
    """Host chain-first strips: [2(w/u), 2(row: ps=-2,-1), 12, 516] fp16.
    piece: [260, 520] f32. Matches device plane values (incl. 2/sqrt(pi))."""
    out = np.zeros((2, 2, NSLOT, PLANE_COLS), dtype=np.float32)
    cols = np.arange(PLANE_COLS)
    for s, (di, dj) in enumerate(REPS):
        for ri, p in enumerate((-2, -1)):
            tap = piece[p + di, cols + dj]
            cen = piece[p + 2, cols + 2]
            d = (tap - cen).astype(np.float32)
            w = NORM * np.exp(-(gamma**2) * d * d)
            out[0, ri, s] = w
            out[1, ri, s] = w * d
    return out.astype(np.float16)


def _get_nc(sk: np.ndarray, gamma: float):
    key = (sk.tobytes(), float(gamma))
    if _cached.get("key") != key:
        wsk_cls = {}
        for s, (di, dj) in enumerate(REPS):
            # fold sqrt(pi)/2 so that wsk * D_ERF = sk * exp(-g^2 d^2)
            wsk_cls[CLS_OF[s]] = float(sk[di, dj]) * float(np.sqrt(np.pi) / 2)
        _cached["key"] = key
        _cached["wsk"] = wsk_cls
        _cached["nc"] = _build(wsk_cls, gamma)
        _cached["lhst"] = _make_lhst(wsk_cls)
    return _cached["nc"], _cached["lhst"]


def kernel(x, spatial_kernel, sigma_color):
    x = np.ascontiguousarray(np.asarray(x, dtype=np.float32))
    sk = np.asarray(spatial_kernel, dtype=np.float64)
    sigma = float(np.asarray(sigma_color))
    gamma = 1.0 / (np.sqrt(2.0) * sigma)

    imgs = x.reshape(N_IMGS, H, W)
    xpad = np.pad(imgs, ((0, 0), (2, 2), (4, 4)), mode="reflect")
    halves_f32 = np.stack(
        [xpad[:, 0:PIECE_ROWS, :], xpad[:, HALF : HALF + PIECE_ROWS, :]], 1
    ).reshape(N_IMGS * 2, PIECE_ROWS, PIECE_COLS)
    halves = halves_f32.astype(np.float16)

    nc, lhst = _get_nc(sk, gamma)

    # core k even: halves [3k, 3k+1, 3k+2]; odd: [3k+1, 3k+2, 3k]
    # (pieces 0,1 always form a full-image chain; piece 2 is a lone chain)
    core_halves = []
    for k in range(N_CORES):
        if k % 2 == 0:
            core_halves.append([3 * k, 3 * k + 1, 3 * k + 2])
        else:
            core_halves.append([3 * k + 1, 3 * k + 2, 3 * k])

    in_maps = []
    for k in range(N_CORES):
        hs = core_halves[k]
        strips = np.stack(
            [_strip_planes(halves_f32[hs[0]], gamma),
             _strip_planes(halves_f32[hs[2]], gamma)]
        )
        in_maps.append({
            "x_in": np.ascontiguousarray(halves[hs]),
            "strips": np.ascontiguousarray(strips),
            "lhst": lhst,
        })

    trace = os.environ.get("BILATERAL_TRACE", "0") == "1"
    res = bass_utils.run_bass_kernel_spmd(
        nc, in_maps, core_ids=list(range(N_CORES)), trace=trace
    )
    kernel.last_results = res

    out = np.empty((N_IMGS * 2, HALF, W), dtype=np.float32)
    for k in range(N_CORES):
        for i, h in enumerate(core_halves[k]):
            out[h] = res.results[k]["y_out"][i]
    return (
        out.reshape(N_IMGS, 2, HALF, W)
        .reshape(N_IMGS, H, W)
        .reshape(B, C, H, W)
        .astype(np.float32)
    )


kernel.last_results = None


# revision 13
# speedup vs baseline: 1.1450x; 1.0696x over previous
"""Bilateral filter (5x5, reflect pad) on 8 Trainium2 NeuronCores.

Contract: kernel(**inputs) takes the FULL inputs
  x:              [4, 3, 512, 512] f32
  spatial_kernel: [5, 5] f32
  sigma_color:    scalar f32
and returns the FULL output [4, 3, 512, 512] f32.

v2: paired-plane algorithm. The bilateral weight between two pixels is
symmetric, so the 24 non-center taps form 12 point-symmetric pairs:
  w_refl[i,j] = w_rep[i-di', j-dj']      (di' = di-2, dj' = dj-2)
  u_refl[i,j] = -u_rep[i-di', j-dj']     (u = w * d, d = tap - center)
Only 12 "representative" planes (di in {3,4} x dj, plus (2,3),(2,4)) are
computed (halves ACT + DVE work vs all 24). Reductions
  S = 1 + sum_t wsk_t (w_t + w_t_shifted)
  U =     sum_t wsk_t (u_t - u_t_shifted)
run on the PE as 51 matmuls/tile: scaled identities for direct terms,
scaled shifted identities (superdiagonal) for row-shifted terms, and two
tiny seam matmuls for the 2 top rows whose shifted reads fall into the
previous tile (strips gathered by SBUF-SBUF DMA from the previous tile's
planes; chain-first tiles get host-computed strips). Output:
  out = center + U * reciprocal(S).

Sharding: each core gets one full image (512-row chain of 4 tiles) plus
one lone half image (256-row chain of 2 tiles) as three [260, 520] fp16
pieces (rows +-2, cols +-4 reflect halo), converted to fp16 on the host.
"""

import os

import numpy as np

import bass_rust
import concourse.bacc as bacc
import concourse.bass as bass
import concourse.mybir as mybir
import concourse.tile as tile
from concourse import bass_utils

F32 = mybir.dt.float32
FP16 = mybir.dt.float16
AF = mybir.ActivationFunctionType
ALU = mybir.AluOpType

N_CORES = 8
K = 5
B, C, H, W = 4, 3, 512, 512
N_IMGS = B * C                    # 12
HALF = 256
PIECE_ROWS = HALF + 4             # 260 (rows +-2)
PIECE_COLS = W + 8                # 520 (cols +-4)
PLANE_COLS = W + 4                # 516
NSLOT = 12

# plane slots: 0..4 = (3, 0..4), 5..9 = (4, 0..4), 10 = (2,3), 11 = (2,4)
REPS = [(3, d) for d in range(5)] + [(4, d) for d in range(5)] + [(2, 3), (2, 4)]
CLS_VALS = [1, 2, 4, 5, 8]        # distinct (di-2)^2 + (dj-2)^2 of reps
CLS_OF = [CLS_VALS.index((di - 2) ** 2 + (dj - 2) ** 2) for di, dj in REPS]

# lhsT pack layout (22 matrices of [128, 128] fp16)
LT_ONES = 0                       # row 0 = ones (center tap: S += 1)
LT_A = 1                          # +wsk_c * I          (5: cls 0..4)
LT_B = 6                          # -wsk_c * I, c in {1, 4} -> cls idx {0, 2}
LT_Z1 = 8                         # +wsk_c * Z1, c in {1,2,5} -> cls {0,1,3}
LT_Z2 = 11                        # +wsk_c * Z2, c in {4,5,8} -> cls {2,3,4}
LT_N1 = 14                        # -wsk_c * Z1
LT_N2 = 17                        # -wsk_c * Z2
LT_SEAM_S = 20
LT_SEAM_U = 21
N_MAT = 22
B_IDX = {0: 6, 2: 7}              # cls idx -> B slot
Z1_IDX = {0: 8, 1: 9, 3: 10}
Z2_IDX = {2: 11, 3: 12, 4: 13}
N1_IDX = {0: 14, 1: 15, 3: 16}
N2_IDX = {2: 17, 3: 18, 4: 19}

SUB_GROUPS = [(3, 0, 0, 5), (4, 0, 5, 5), (2, 3, 10, 2)]  # (di, dj0, slot0, g)
USE_VZ = True

PLANE_EXT = NSLOT * PLANE_COLS    # 6192 elems per partition in W/U planes
NORM = float(2.0 / np.sqrt(np.pi))   # Derivative_Erf amplitude

_cached = {}


def _build(wsk_cls: dict, gamma: float) -> bass.Bass:
    """Per-core Bass module (SPMD: same NEFF on all 8 cores)."""
    nc = bacc.Bacc("TRN2", target_bir_lowering=False, debug=False)
    x_in = nc.dram_tensor(
        "x_in", [3, PIECE_ROWS, PIECE_COLS], FP16, kind="ExternalInput"
    ).ap()
    strips_in = nc.dram_tensor(
        "strips", [2, 2, 2, NSLOT, PLANE_COLS], FP16, kind="ExternalInput"
    ).ap()
    lhst_in = nc.dram_tensor(
        "lhst", [128, N_MAT, 128], FP16, kind="ExternalInput"
    ).ap()
    y_out = nc.dram_tensor(
        "y_out", [3, HALF, W], F32, kind="ExternalOutput"
    ).ap()

    with tile.TileContext(nc) as tc:
        with (
            tc.tile_pool(name="const_pool", bufs=1) as const_pool,
            tc.tile_pool(name="slab_pool", bufs=2) as slab_pool,
            tc.tile_pool(name="d_pool", bufs=2) as d_pool,
            tc.tile_pool(name="w_pool", bufs=2) as w_pool,
            tc.tile_pool(name="u_pool", bufs=2) as u_pool,
            tc.tile_pool(name="seam_pool", bufs=2) as seam_pool,
            tc.tile_pool(name="epi_pool", bufs=2) as epi_pool,
            tc.tile_pool(name="psum_pool", bufs=2, space="PSUM") as psum_pool,
        ):
            lhst = const_pool.tile([128, N_MAT, 128], FP16, tag="lhst",
                                   name="lhst")
            nc.sync.dma_start(lhst[:, :, :], lhst_in)
            ones_row = const_pool.tile([1, W], FP16, tag="ones", name="ones")
            nc.gpsimd.memset(ones_row[:, :], 1.0)

            # PE warmup: dummy matmuls depending only on the memset keep the
            # PE HAM busy through the first tile's plane production, so the
            # real burst starts at full clock (2.4 GHz, not cold 1.2).
            warm_ps = psum_pool.tile([128, W], F32, tag="warm", name="warm")
            for wi in range(0):
                nc.tensor.matmul(warm_ps[:, :], ones_row[0:1, 0:128],
                                 ones_row[0:1, :], start=(wi == 0),
                                 stop=(wi == 13))

            # tiles: (piece, r0, chain_first, chain_idx)
            tiles = [
                (0, 0, True, 0), (0, 128, False, 0),
                (1, 0, False, 0), (1, 128, False, 0),
                (2, 0, True, 1), (2, 128, False, 1),
            ]
            st = {}

            def prod(ti):
                """Plane production for tile ti: slab DMA, seam gathers,
                d -> w -> u, col-pair presums."""
                pc, r0, first, ch = tiles[ti]
                slab = slab_pool.tile([128, 3, PIECE_COLS], FP16, tag="slab",
                                      name=f"slab{ti}")
                src = x_in[pc, r0 + 2 : r0 + 2 + 128, :].copy()
                src.ap = bass_rust.VecI64Pair(
                    [(PIECE_COLS, 128), (PIECE_COLS, 3), (1, PIECE_COLS)]
                )
                nc.sync.dma_start(slab[:, :, :], src)
                slab_base = slab[:, :, :].offset

                seamW = seam_pool.tile([15, W], FP16, tag="sw", name=f"sw{ti}")
                seamU = seam_pool.tile([15, W], FP16, tag="su", name=f"su{ti}")
                for dst, which in ((seamW, 0), (seamU, 1)):
                    if first:
                        base = ch * (2 * 2 * PLANE_EXT) + which * (2 * PLANE_EXT)
                        proto = strips_in
                        off0 = base + 1 * PLANE_EXT + 4          # row ps=-1
                        off1 = base + 0 * PLANE_EXT + 5 * PLANE_COLS + 4
                    else:
                        prev = st[ti - 1]["W"] if which == 0 else st[ti - 1]["U"]
                        proto = prev[:, :, :]
                        pbase = proto.offset
                        off0 = pbase + 127 * PLANE_EXT + 4       # prev row 127
                        off1 = pbase + 126 * PLANE_EXT + 5 * PLANE_COLS + 4
                    v = proto.copy()
                    v.ap = bass_rust.VecI64Pair(
                        [(PLANE_EXT, 1), (PLANE_COLS - 1, 5), (1, W)]
                    )
                    v.offset = off0
                    nc.sync.dma_start(dst[0:5, :], v)
                    v2 = proto.copy()
                    v2.ap = bass_rust.VecI64Pair(
                        [(PLANE_EXT, 2), (PLANE_COLS - 1, 5), (1, W)]
                    )
                    v2.offset = off1
                    nc.sync.dma_start(dst[5:15, :], v2)

                d_all = d_pool.tile([128, NSLOT, PLANE_COLS], FP16, tag="d",
                                    name=f"d{ti}")
                W_all = w_pool.tile([128, NSLOT, PLANE_COLS], FP16, tag="w",
                                    name=f"w{ti}")
                U_all = u_pool.tile([128, NSLOT, PLANE_COLS], FP16, tag="u",
                                    name=f"u{ti}")
                for di, dj0, s0, g in SUB_GROUPS:
                    tap = slab[:, :, :].copy()
                    tap.ap = bass_rust.VecI64Pair(
                        [(3 * PIECE_COLS, 128), (1, g), (1, PLANE_COLS)]
                    )
                    tap.offset = slab_base + (di - 2) * PIECE_COLS + dj0
                    cen = slab[:, :, :].copy()
                    cen.ap = bass_rust.VecI64Pair(
                        [(3 * PIECE_COLS, 128), (0, g), (1, PLANE_COLS)]
                    )
                    cen.offset = slab_base + 2
                    nc.vector.tensor_sub(d_all[:, s0 : s0 + g, :], tap, cen)
                    nc.scalar.activation(W_all[:, s0 : s0 + g, :],
                                         d_all[:, s0 : s0 + g, :],
                                         AF.Derivative_Erf, scale=float(gamma))
                    nc.vector.tensor_mul(U_all[:, s0 : s0 + g, :],
                                         W_all[:, s0 : s0 + g, :],
                                         d_all[:, s0 : s0 + g, :])

                vt = zt = None
                if not USE_VZ:
                    st[ti] = dict(slab=slab, slab_base=slab_base, seamW=seamW,
                                  seamU=seamU, W=W_all, U=U_all, vt=vt, zt=zt)
                    return
                vt = seam_pool.tile([128, 2, W], FP16, tag="v", name=f"v{ti}")
                zt = seam_pool.tile([128, 2, W], FP16, tag="z", name=f"z{ti}")
                wsh = W_all[:, :, :].copy()
                wsh.ap = bass_rust.VecI64Pair(
                    [(PLANE_EXT, 128), (PLANE_COLS - 1, 2), (1, W)]
                )
                wsh.offset = W_all[:, :, :].offset + 10 * PLANE_COLS + 1
                nc.vector.tensor_add(vt[:, :, :], W_all[:, 10:12, 2 : 2 + W],
                                     wsh)
                ush = U_all[:, :, :].copy()
                ush.ap = bass_rust.VecI64Pair(
                    [(PLANE_EXT, 128), (PLANE_COLS - 1, 2), (1, W)]
                )
                ush.offset = U_all[:, :, :].offset + 10 * PLANE_COLS + 1
                nc.vector.tensor_sub(zt[:, :, :], U_all[:, 10:12, 2 : 2 + W],
                                     ush)
                st[ti] = dict(slab=slab, slab_base=slab_base, seamW=seamW,
                              seamU=seamU, W=W_all, U=U_all, vt=vt, zt=zt)

            def burst_epi(ti):
                pc, r0, first, ch = tiles[ti]
                s_ = st[ti]
                W_all, U_all = s_["W"], s_["U"]
                S_ps = psum_pool.tile([128, W], F32, tag="S", name=f"S{ti}")
                U_ps = psum_pool.tile([128, W], F32, tag="U", name=f"U{ti}")
                nc.tensor.matmul(S_ps[:, :], lhst[0:15, LT_SEAM_S, :],
                                 s_["seamW"][0:15, :], start=True, stop=False)
                nc.tensor.matmul(S_ps[:, :], lhst[0:1, LT_ONES, :],
                                 ones_row[0:1, :], start=False, stop=False)
                nc.tensor.matmul(U_ps[:, :], lhst[0:15, LT_SEAM_U, :],
                                 s_["seamU"][0:15, :], start=True, stop=False)
                for s in range(10):
                    c = CLS_OF[s]
                    a = 4 - REPS[s][1]
                    sshift = Z1_IDX[c] if s < 5 else Z2_IDX[c]
                    ushift = N1_IDX[c] if s < 5 else N2_IDX[c]
                    nc.tensor.matmul(S_ps[:, :], lhst[:, LT_A + c, :],
                                     W_all[:, s, 2 : 2 + W],
                                     start=False, stop=False)
                    nc.tensor.matmul(U_ps[:, :], lhst[:, LT_A + c, :],
                                     U_all[:, s, 2 : 2 + W],
                                     start=False, stop=False)
                    nc.tensor.matmul(S_ps[:, :], lhst[:, sshift, :],
                                     W_all[:, s, a : a + W],
                                     start=False, stop=False)
                    nc.tensor.matmul(U_ps[:, :], lhst[:, ushift, :],
                                     U_all[:, s, a : a + W],
                                     start=False, stop=False)
                if USE_VZ:
                    for i, s in enumerate((10, 11)):
                        c = CLS_OF[s]
                        nc.tensor.matmul(S_ps[:, :], lhst[:, LT_A + c, :],
                                         s_["vt"][:, i, :], start=False,
                                         stop=(s == 11))
                        nc.tensor.matmul(U_ps[:, :], lhst[:, LT_A + c, :],
                                         s_["zt"][:, i, :], start=False,
                                         stop=(s == 11))
                else:
                    for s in (10, 11):
                        c = CLS_OF[s]
                        a = 4 - REPS[s][1]
                        nc.tensor.matmul(S_ps[:, :], lhst[:, LT_A + c, :],
                                         W_all[:, s, 2 : 2 + W],
                                         start=False, stop=False)
                        nc.tensor.matmul(U_ps[:, :], lhst[:, LT_A + c, :],
                                         U_all[:, s, 2 : 2 + W],
                                         start=False, stop=False)
                        nc.tensor.matmul(S_ps[:, :], lhst[:, LT_A + c, :],
                                         W_all[:, s, a : a + W],
                                         start=False, stop=(s == 11))
                        nc.tensor.matmul(U_ps[:, :], lhst[:, B_IDX[c], :],
                                         U_all[:, s, a : a + W],
                                         start=False, stop=(s == 11))

                R = epi_pool.tile([128, W], F32, tag="R", name=f"R{ti}")
                nc.vector.reciprocal_approx_fast(R[:, :], S_ps[:, :])
                UR = epi_pool.tile([128, W], F32, tag="UR", name=f"UR{ti}")
                nc.vector.scalar_tensor_tensor(UR[:, :], U_ps[:, :], 1.0,
                                               R[:, :], ALU.mult, ALU.mult)
                out_t = epi_pool.tile([128, W], F32, tag="out", name=f"o{ti}")
                cen = s_["slab"][:, :, :].copy()
                cen.ap = bass_rust.VecI64Pair([(3 * PIECE_COLS, 128), (1, W)])
                cen.offset = s_["slab_base"] + 4
                nc.vector.tensor_add(out_t[:, :], UR[:, :], cen)
                nc.sync.dma_start(y_out[pc, r0 : r0 + 128, :], out_t[:, :])

            # software-pipelined emission: production runs one tile ahead
            prod(0)
            for ti in range(6):
                if ti + 1 < 6:
                    prod(ti + 1)
                burst_epi(ti)
    nc.compile()
    return nc


def _make_lhst(wsk_cls: dict) -> np.ndarray:
    """[128, N_MAT, 128] fp16 lhsT pack."""
    mats = np.zeros((N_MAT, 128, 128), dtype=np.float32)
    eye = np.eye(128, dtype=np.float32)
    z1 = np.zeros((128, 128), dtype=np.float32)
    z1[np.arange(127), np.arange(1, 128)] = 1.0   # Z1[p, p+1] = 1
    z2 = np.zeros((128, 128), dtype=np.float32)
    z2[np.arange(126), np.arange(2, 128)] = 1.0
    mats[LT_ONES, 0, :] = 1.0
    for ci in range(5):
        mats[LT_A + ci] = wsk_cls[ci] * eye
    for ci, sl in B_IDX.items():
        mats[sl] = -wsk_cls[ci] * eye
    for ci, sl in Z1_IDX.items():
        mats[sl] = wsk_cls[ci] * z1
    for ci, sl in Z2_IDX.items():
        mats[sl] = wsk_cls[ci] * z2
    for ci, sl in N1_IDX.items():
        mats[sl] = -wsk_cls[ci] * z1
    for ci, sl in N2_IDX.items():
        mats[sl] = -wsk_cls[ci] * z2
    # seam lhsTs: rows 0..4 (dlt_i=1 pairs, slots 0..4) -> out row 0;
    # rows 5..9 (dlt_i=2, ps=-2) -> out row 0; rows 10..14 (ps=-1) -> row 1
    for r in range(5):
        mats[LT_SEAM_S, r, 0] = wsk_cls[CLS_OF[r]]
        mats[LT_SEAM_U, r, 0] = -wsk_cls[CLS_OF[r]]
        mats[LT_SEAM_S, 5 + r, 0] = wsk_cls[CLS_OF[5 + r]]
        mats[LT_SEAM_U, 5 + r, 0] = -wsk_cls[CLS_OF[5 + r]]
        mats[LT_SEAM_S, 10 + r, 1] = wsk_cls[CLS_OF[5 + r]]
        mats[LT_SEAM_U, 10 + r, 1] = -wsk_cls[CLS_OF[5 + r]]
    return np.ascontiguousarray(
        np.transpose(mats, (1, 0, 2)).astype(np.float16)
    )


def _strip_planes(piece: np.ndarray, gamma: float) -> np.ndarray:
    """Host chain-first strips: [2(w/u), 2(row: ps=-2,-1), 12, 516] fp16.
    piece: [260, 520] f32. Matches device plane values (incl. 2/sqrt(pi))."""
    out = np.zeros((2, 2, NSLOT, PLANE_COLS), dtype=np.float32)
    cols = np.arange(PLANE_COLS)
    for s, (di, dj) in enumerate(REPS):
        for ri, p in enumerate((-2, -1)):
            tap = piece[p + di, cols + dj]
            cen = piece[p + 2, cols + 2]
            d = (tap - cen).astype(np.float32)
            w = NORM * np.exp(-(gamma**2) * d * d)
            out[0, ri, s] = w
            out[1, ri, s] = w * d
    return out.astype(np.float16)


def _get_nc(sk: np.ndarray, gamma: float):
    key = (sk.tobytes(), float(gamma))
    if _cached.get("key") != key:
        wsk_cls = {}
        for s, (di, dj) in enumerate(REPS):
            # fold sqrt(pi)/2 so that wsk * D_ERF = sk * exp(-g^2 d^2)
            wsk_cls[CLS_OF[s]] = float(sk[di, dj]) * float(np.sqrt(np.pi) / 2)
        _cached["key"] = key
        _cached["wsk"] = wsk_cls
        _cached["nc"] = _build(wsk_cls, gamma)
        _cached["lhst"] = _make_lhst(wsk_cls)
    return _cached["nc"], _cached["lhst"]


def kernel(x, spatial_kernel, sigma_color):
    x = np.ascontiguousarray(np.asarray(x, dtype=np.float32))
    sk = np.asarray(spatial_kernel, dtype=np.float64)
    sigma = float(np.asarray(sigma_color))
    gamma = 1.0 / (np.sqrt(2.0) * sigma)

    imgs = x.reshape(N_IMGS, H, W)
    xpad = np.pad(imgs, ((0, 0), (2, 2), (4, 4)), mode="reflect")
    halves_f32 = np.stack(
        [xpad[:, 0:PIECE_ROWS, :], xpad[:, HALF : HALF + PIECE_ROWS, :]], 1
    ).reshape(N_IMGS * 2, PIECE_ROWS, PIECE_COLS)
    halves = halves_f32.astype(np.float16)

    nc, lhst = _get_nc(sk, gamma)

    # core k even: halves [3k, 3k+1, 3k+2]; odd: [3k+1, 3k+2, 3k]
    # (pieces 0,1 always form a full-image chain; piece 2 is a lone chain)
    core_halves = []
    for k in range(N_CORES):
        if k % 2 == 0:
            core_halves.append([3 * k, 3 * k + 1, 3 * k + 2])
        else:
            core_halves.append([3 * k + 1, 3 * k + 2, 3 * k])

    in_maps = []
    for k in range(N_CORES):
        hs = core_halves[k]
        strips = np.stack(
            [_strip_planes(halves_f32[hs[0]], gamma),
             _strip_planes(halves_f32[hs[2]], gamma)]
        )
        in_maps.append({
            "x_in": np.ascontiguousarray(halves[hs]),
            "strips": np.ascontiguousarray(strips),
            "lhst": lhst,
        })

    trace = os.environ.get("BILATERAL_TRACE", "0") == "1"
    res = bass_utils.run_bass_kernel_spmd(
        nc, in_maps, core_ids=list(range(N_CORES)), trace=trace
    )
    kernel.last_results = res

    out = np.empty((N_IMGS * 2, HALF, W), dtype=np.float32)
    for k in range(N_CORES):
        for i, h in enumerate(core_halves[k]):
            out[h] = res.results[k]["y_out"][i]
    return (
        out.reshape(N_IMGS, 2, HALF, W)
        .reshape(N_IMGS, H, W)
        .reshape(B, C, H, W)
        .astype(np.float32)
    )


kernel.last_results = None
